# revision 1
# baseline (speedup 1.0000x reference)
"""DCT patch denoiser on 8 Trainium2 NeuronCores.

Sharding: data-parallel over (image, top/bottom half) = 8 shards.
Each core: unfold -> DCT (P^T @ patches, fp32r matmuls) -> hardshrink +
AC-nonzero count -> w = 1/(1+count) -> inverse DCT -> recon (bf16) to
DRAM -> diagonal-AP re-gather -> ones-matmul overlap-add fold -> canvas.
The divisor plane (fold of w) and final division happen on host from the
returned per-patch weights.
"""

import os
import sys
import numpy as np

for _p in ("/opt/trn_rl_repo",):
    if _p not in sys.path:
        sys.path.insert(0, _p)

import ml_dtypes  # noqa: E402

# ---- hardcoded problem geometry ----
PATCH = 16
H = W = 256
Ho = Wo = H - PATCH + 1          # 241
Wp = 256                          # padded patch-col count (j in [0,256))
NROWS = 122                       # local patch rows per core (incl masked)
NIN = 138                         # input rows per core
NPAIR = NROWS // 2                # 61 main tiles
FPAIR = 69                        # fold row-pairs -> canvas rows 0..137
PADL = 16                         # head pad elems in recon rows
RSLOT = 153                       # recon row slots (rp+15) in [0,152]
RSTRIDE = PADL + RSLOT * Wp       # per-feature stride in recon buffer

_CACHE = {}
LAST_EXEC_NS = None


def _build_dct_matrix(p):
    x = np.arange(p)[:, None]
    i = np.arange(p)[None, :]
    A = np.sqrt(2.0 / p) * np.cos((2 * x + 1) * i * np.pi / (2 * p))
    A[:, 0] /= np.sqrt(2.0)
    return np.kron(A, A).astype(np.float32)


def _build_program(thr):
    import concourse.bass as bass
    import concourse.mybir as mybir
    import concourse.tile as tile
    from concourse import bacc
    from contextlib import ExitStack

    dt = mybir.dt
    f32, f32r, bf16 = dt.float32, dt.float32r, dt.bfloat16
    Alu = mybir.AluOpType

    nc = bacc.Bacc("TRN2", target_bir_lowering=False, debug=False)
    ximg = nc.dram_tensor("ximg", [NIN * 256], f32r, kind="ExternalInput").ap()
    pfwd = nc.dram_tensor("pfwd", [2, 128, 256], f32r, kind="ExternalInput").ap()
    pinv = nc.dram_tensor("pinv", [2, 128, 256], bf16, kind="ExternalInput").ap()
    onesac = nc.dram_tensor("onesac", [2, 128, 1], bf16, kind="ExternalInput").ap()
    wmaskd = nc.dram_tensor("wmask", [NROWS * 256], f32, kind="ExternalInput").ap()
    zerosd = nc.dram_tensor("zeros", [128, 4096], bf16, kind="ExternalInput").ap()
    onesk = nc.dram_tensor("onesk", [1, 128], bf16, kind="ExternalInput").ap()
    onesr = nc.dram_tensor("onesr", [1, 512], bf16, kind="ExternalInput").ap()
    canvas = nc.dram_tensor("canvas", [FPAIR * 512], f32, kind="ExternalOutput").ap()
    woutd = nc.dram_tensor("wout", [NROWS * 256], bf16, kind="ExternalOutput").ap()
    recon = nc.dram_tensor("recon", [256 * RSTRIDE], bf16)

    xh = ximg.tensor
    rh = recon[:].tensor

    with tile.TileContext(nc) as tc:
        with ExitStack() as ctx:
            const = ctx.enter_context(tc.tile_pool(name="const", bufs=1))
            pf = [const.tile([128, 256], f32r, tag=f"pf{h}", name=f"pf{h}") for h in range(2)]
            pi = [const.tile([128, 256], bf16, tag=f"pi{h}", name=f"pi{h}") for h in range(2)]
            oa = [const.tile([128, 1], bf16, tag=f"oa{h}", name=f"oa{h}") for h in range(2)]
            ok1 = const.tile([1, 128], bf16, tag="ok1", name="ok1")
            okr = const.tile([1, 512], bf16, tag="okr", name="okr")
            onesb = const.tile([128, 1], bf16, tag="onesb", name="onesb")
            for h in range(2):
                nc.sync.dma_start(out=pf[h][:], in_=pfwd[h])
                nc.sync.dma_start(out=pi[h][:], in_=pinv[h])
                nc.sync.dma_start(out=oa[h][:], in_=onesac[h])
            nc.sync.dma_start(out=ok1[:], in_=onesk)
            nc.sync.dma_start(out=okr[:], in_=onesr)
            nc.sync.dma_start(out=onesb[:], in_=onesk.rearrange("a b -> b a"))
            # zero recon pad regions
            for h in range(2):
                base = h * 128 * RSTRIDE
                out_ap = bass.AP(tensor=rh, offset=base,
                                 ap=[[RSTRIDE, 128], [1, 3856]])
                nc.sync.dma_start(out=out_ap, in_=zerosd[:, :3856])
                out_ap = bass.AP(tensor=rh, offset=base + PADL + 137 * 256,
                                 ap=[[RSTRIDE, 128], [1, 4096]])
                nc.sync.dma_start(out=out_ap, in_=zerosd[:, :4096])

            sb = ctx.enter_context(tc.tile_pool(name="sb", bufs=4))
            st = ctx.enter_context(tc.tile_pool(name="st", bufs=2))
            sk = ctx.enter_context(tc.tile_pool(name="sk", bufs=4))
            fg = ctx.enter_context(tc.tile_pool(name="fg", bufs=6))
            psc = ctx.enter_context(tc.tile_pool(name="psc", bufs=3, space="PSUM"))
            psm = ctx.enter_context(tc.tile_pool(name="psm", bufs=1, space="PSUM"))
            psr = ctx.enter_context(tc.tile_pool(name="psr", bufs=2, space="PSUM"))
            psf = ctx.enter_context(tc.tile_pool(name="psf", bufs=1, space="PSUM"))

            fold_state = {"cv": None, "base": 0}

            def fold_flush(upto):
                if fold_state["cv"] is not None:
                    b = fold_state["base"]
                    nc.sync.dma_start(
                        out=canvas[None, b * 512:upto * 512],
                        in_=fold_state["cv"][:, :(upto - b) * 512])
                    fold_state["cv"] = None

            def fold_pair(tt):
                if fold_state["cv"] is None:
                    fold_state["cv"] = st.tile([1, 8 * 512], f32, tag="cv",
                                               name="cv")
                    fold_state["base"] = tt
                pF = psf.tile([1, 512], f32, tag="psF", name="psF")
                for h in range(2):
                    g = fg.tile([128, 512], bf16, tag=f"g{h}", name=f"g{h}")
                    in_ap = bass.AP(
                        tensor=rh,
                        offset=h * 128 * RSTRIDE + PADL
                        + (2 * tt + 15 - 8 * h) * 256,
                        ap=[[16 * RSTRIDE - 256, 8], [RSTRIDE - 1, 16],
                            [1, 512]])
                    nc.gpsimd.dma_start(out=g[:], in_=in_ap)
                    nc.tensor.matmul(pF[:], lhsT=onesb[:, 0:1], rhs=g[:],
                                     start=(h == 0), stop=(h == 1))
                off = (tt - fold_state["base"]) * 512
                nc.scalar.copy(out=fold_state["cv"][:, off:off + 512], in_=pF[:])
                if tt - fold_state["base"] == 7 or tt == FPAIR - 1:
                    fold_flush(tt + 1)

            wstate = {"wmc": None, "woc": None, "base": 0}
            for t in range(NPAIR):
                pat = []
                for h in range(2):
                    ptile = sb.tile([128, 512], f32r, tag=f"pat{h}", name=f"pat{h}")
                    in_ap = bass.AP(
                        tensor=xh, offset=(2 * t + 8 * h) * 256,
                        ap=[[256, 8], [1, 16], [1, 512]])
                    nc.scalar.dma_start(out=ptile[:], in_=in_ap)
                    pat.append(ptile)
                # forward DCT: coeffs[k,l], two k-chunks
                psC = []
                for m in range(2):
                    pc = psc.tile([128, 512], f32, tag="psC", name=f"psC{m}")
                    for h in range(2):
                        nc.tensor.matmul(
                            pc[:],
                            lhsT=pf[h][:, m * 128:(m + 1) * 128],
                            rhs=pat[h][:],
                            start=(h == 0), stop=(h == 1))
                    psC.append(pc)
                # |coeffs| on ACT, indicator on GPSIMD (bf16)
                ind = []
                ab = []
                for m in range(2):
                    a_m = sk.tile([128, 512], f32, tag=f"ab{m}", name=f"ab{m}")
                    nc.scalar.activation(out=a_m[:], in_=psC[m][:],
                                         func=mybir.ActivationFunctionType.Abs)
                    ab.append(a_m)
                    it = sk.tile([128, 512], bf16, tag=f"ind{m}", name=f"ind{m}")
                    nc.gpsimd.tensor_scalar(
                        out=it[:], in0=a_m[:], scalar1=thr, scalar2=None,
                        op0=Alu.is_gt)
                    ind.append(it)
                # count (+1 seed): psN = 1 + sum_ac ind
                pN = psm.tile([1, 512], f32, tag="psN", name="psN")
                nc.tensor.matmul(pN[:], lhsT=ok1[:, 0:1], rhs=okr[:],
                                 start=True, stop=False)
                for m in range(2):
                    nc.tensor.matmul(
                        pN[:], lhsT=oa[m][:, 0:1], rhs=ind[m][:],
                        start=False, stop=(m == 1))
                # w row = mask * 1/(1+count)
                if t % 8 == 0:
                    wmc = st.tile([1, 8 * 512], f32, tag="wmc", name="wmc")
                    nend = min((t + 8) * 512, NROWS * 256)
                    nc.sync.dma_start(out=wmc[:, :nend - t * 512],
                                      in_=wmaskd[None, t * 512:nend])
                    woc = st.tile([1, 8 * 512], bf16, tag="woc", name="woc")
                    wstate["wmc"], wstate["woc"], wstate["base"] = wmc, woc, t
                wr = sk.tile([1, 512], f32, tag="wr", name="wr")
                nc.vector.reciprocal(out=wr[:], in_=pN[:])
                woff = (t - wstate["base"]) * 512
                wf = wstate["woc"][:, woff:woff + 512]
                nc.vector.scalar_tensor_tensor(
                    out=wf, in0=wr[:], scalar=1.0,
                    in1=wstate["wmc"][:, woff:woff + 512],
                    op0=Alu.mult, op1=Alu.mult)
                if t % 8 == 7 or t == NPAIR - 1:
                    nc.sync.dma_start(
                        out=woutd[None, wstate["base"] * 512:(t + 1) * 512],
                        in_=wstate["woc"][:, :woff + 512])
                wbp = psm.tile([128, 512], f32, tag="wbp", name="wbp")
                nc.tensor.matmul(wbp[:], lhsT=ok1[:], rhs=wf,
                                 start=True, stop=True)
                wbs = sk.tile([128, 512], f32, tag="wbs", name="wbs")
                nc.scalar.copy(out=wbs[:], in_=wbp[:])
                # shrunk = coeffs * ind
                vv = []
                for m in range(2):
                    vt = sk.tile([128, 512], bf16, tag=f"v{m}", name=f"v{m}")
                    nc.vector.scalar_tensor_tensor(
                        out=vt[:], in0=psC[m][:], scalar=0.0, in1=ind[m][:],
                        op0=Alu.add, op1=Alu.mult)
                    vv.append(vt)
                # inverse DCT + w-scaled bf16 evacuation + writeback
                for h in range(2):
                    pr = psr.tile([128, 512], f32, tag="psR", name=f"psR{h}")
                    for m in range(2):
                        nc.tensor.matmul(
                            pr[:],
                            lhsT=pi[m][:, h * 128:(h + 1) * 128],
                            rhs=vv[m][:],
                            start=(m == 0), stop=(m == 1))
                    rb = sk.tile([128, 512], bf16, tag=f"rb{h}", name=f"rb{h}")
                    nc.vector.tensor_tensor(out=rb[:], in0=pr[:], in1=wbs[:],
                                            op=Alu.mult)
                    out_ap = bass.AP(
                        tensor=rh,
                        offset=h * 128 * RSTRIDE + PADL + (2 * t + 15) * 256,
                        ap=[[RSTRIDE, 128], [1, 512]])
                    nc.gpsimd.dma_start(out=out_ap, in_=rb[:])
                if t >= 9:
                    fold_pair(t - 9)
            for tt in range(NPAIR - 9, FPAIR):
                fold_pair(tt)


    nc.compile()
    return nc


def _prep_inputs(x, Pm):
    """Per-core input maps."""
    Pm = np.ascontiguousarray(Pm, dtype=np.float32)
    pfwd = np.stack([Pm[0:128], Pm[128:256]])               # lhsT fwd [f,k]
    Pt = np.ascontiguousarray(Pm.T)
    pinv = np.stack([Pt[0:128], Pt[128:256]]).astype(ml_dtypes.bfloat16)
    onesac = np.ones((2, 128, 1), ml_dtypes.bfloat16)
    onesac[0, 0, 0] = 0.0
    in_maps = []
    for core in range(8):
        n, half = core // 2, core % 2
        r0 = 0 if half == 0 else 120
        ximg = np.zeros((NIN, 256), np.float32)
        src = x[n, 0, r0:min(r0 + NIN, 256)]
        ximg[: src.shape[0]] = src
        wmask = np.zeros((NROWS, 256), np.float32)
        if half == 0:
            wmask[0:120, :Wo] = 1.0
        else:
            wmask[0:121, :Wo] = 1.0
        in_maps.append({
            "ximg": ximg.reshape(-1),
            "pfwd": pfwd, "pinv": pinv, "onesac": onesac,
            "wmask": wmask.reshape(-1),
            "zeros": np.zeros((128, 4096), ml_dtypes.bfloat16),
            "onesk": np.ones((1, 128), ml_dtypes.bfloat16),
            "onesr": np.ones((1, 512), ml_dtypes.bfloat16),
        })
    return in_maps


def _assemble(results, x):
    N = x.shape[0]
    out = np.zeros((N, 256, 256), np.float32)
    wplane = np.zeros((N, 256, 256), np.float32)
    for core in range(8):
        n, half = core // 2, core % 2
        r0 = 0 if half == 0 else 120
        canvas = np.asarray(results[core]["canvas"], np.float32).reshape(-1, 256)
        wout = np.asarray(results[core]["wout"]).astype(np.float32).reshape(NROWS, 256)
        rows = min(canvas.shape[0], 256 - r0)
        out[n, r0:r0 + rows] += canvas[:rows]
        prow = min(NROWS, Ho - r0)
        wplane[n, r0:r0 + prow, :Wo] += wout[:prow, :Wo]
    # divisor: 16x16 box-filter of wplane via 2D cumsum
    cp = np.zeros((N, 257, 257), np.float32)
    cp[:, 1:, 1:] = np.cumsum(np.cumsum(wplane, axis=1), axis=2)
    r1 = np.arange(256) + 1
    r0_ = np.maximum(r1 - PATCH, 0)
    div = (cp[:, r1][:, :, r1] - cp[:, r0_][:, :, r1]
           - cp[:, r1][:, :, r0_] + cp[:, r0_][:, :, r0_])
    return (out / div).reshape(N, 1, 256, 256).astype(np.float32)


def kernel(x, P=None, sigma=None, **_unused):
    from concourse.bass_utils import run_bass_kernel_spmd

    x = np.asarray(x, dtype=np.float32)
    if P is None:
        P = _build_dct_matrix(PATCH)
    P = np.asarray(P, dtype=np.float32)
    sig = float(np.float32(sigma)) if sigma is not None else 0.1
    thr = float(np.float32(3.0) * np.float32(sig))

    key = ("prog", thr)
    if key not in _CACHE:
        _CACHE[key] = _build_program(thr)
    nc = _CACHE[key]

    in_maps = _prep_inputs(x, P)
    trace = os.environ.get("DCT_TRACE") == "1"
    res = run_bass_kernel_spmd(nc, in_maps, list(range(8)), trace=trace)
    global LAST_EXEC_NS
    if res.exec_time_ns is not None:
        LAST_EXEC_NS = res.exec_time_ns
    return _assemble(res.results, x)


if __name__ == "__main__":
    import reference
    inputs = reference.setup_inputs()
    expected = np.asarray(reference.reference(**inputs))
    actual = kernel(**{k: np.asarray(v) for k, v in inputs.items()})
    d = actual - expected
    print("l2 rel:", np.linalg.norm(d) / np.linalg.norm(expected))
    print("max abs:", np.abs(d).max())



# revision 2
# speedup vs baseline: 1.1573x; 1.1573x over previous
"""DCT patch denoiser on 8 Trainium2 NeuronCores.

Sharding: data-parallel over (image, top/bottom half) = 8 shards.
Per core: band-deduped unfold loads (fp16) -> forward DCT (fp16 matmuls)
-> fused |c|>thr indicator (abs_max+is_gt tensor_scalar) -> count via
ones-matmuls seeded with a mask row (1 valid / 1e30 invalid) -> w =
reciprocal (bf16) -> broadcast via K=1 matmul -> shrunk coeffs (stt) ->
inverse DCT (bf16) -> rb = psR * w (DVE) -> recon planes in DRAM ->
batched diagonal-AP re-gather -> ones-matmul overlap-add fold -> canvas.
Divisor plane (fold of w) and final division happen on host from wout.
"""

import os
import sys
import numpy as np

for _p in ("/opt/trn_rl_repo",):
    if _p not in sys.path:
        sys.path.insert(0, _p)

import ml_dtypes  # noqa: E402

# ---- hardcoded problem geometry ----
PATCH = 16
H = W = 256
Ho = Wo = H - PATCH + 1          # 241
NROWS = 122                       # local patch rows per core (incl masked)
NIN = 138                         # input rows per core
NPAIR = NROWS // 2                # 61 main tiles
FPAIR = 69                        # fold row-pairs -> canvas rows 0..137
PADL = 16                         # head pad elems in recon rows
RSLOT = 153                       # recon row slots (rp+15) in [0,152]
RSTRIDE = PADL + RSLOT * 256      # per-feature stride in recon buffer
NBAND = 65                        # deduped 8-row bands per core
NGRP = 17                         # band groups of <=4

_CACHE = {}
LAST_EXEC_NS = None


def _build_dct_matrix(p):
    x = np.arange(p)[:, None]
    i = np.arange(p)[None, :]
    A = np.sqrt(2.0 / p) * np.cos((2 * x + 1) * i * np.pi / (2 * p))
    A[:, 0] /= np.sqrt(2.0)
    return np.kron(A, A).astype(np.float32)


def _build_program(thr):
    import concourse.bass as bass
    import concourse.mybir as mybir
    import concourse.tile as tile
    from concourse import bacc
    from contextlib import ExitStack

    dt = mybir.dt
    f32, bf16, f16 = dt.float32, dt.bfloat16, dt.float16
    Alu = mybir.AluOpType

    nc = bacc.Bacc("TRN2", target_bir_lowering=False, debug=False)
    ximg = nc.dram_tensor("ximg", [NIN * 256], f16, kind="ExternalInput").ap()
    pfwd = nc.dram_tensor("pfwd", [2, 128, 256], f16, kind="ExternalInput").ap()
    pinv = nc.dram_tensor("pinv", [2, 128, 256], bf16, kind="ExternalInput").ap()
    onesac = nc.dram_tensor("onesac", [2, 128, 1], bf16, kind="ExternalInput").ap()
    seedd = nc.dram_tensor("seedd", [NROWS * 256], bf16, kind="ExternalInput").ap()
    zerosd = nc.dram_tensor("zeros", [128, 4096], bf16, kind="ExternalInput").ap()
    onesk = nc.dram_tensor("onesk", [1, 128], bf16, kind="ExternalInput").ap()
    canvas = nc.dram_tensor("canvas", [FPAIR * 512], f32, kind="ExternalOutput").ap()
    woutd = nc.dram_tensor("wout", [NROWS * 256], bf16, kind="ExternalOutput").ap()
    recon = nc.dram_tensor("recon", [256 * RSTRIDE], bf16)

    xh = ximg.tensor
    rh = recon[:].tensor

    with tile.TileContext(nc) as tc:
        with ExitStack() as ctx:
            const = ctx.enter_context(tc.tile_pool(name="const", bufs=1))
            pf = [const.tile([128, 256], f16, tag=f"pf{h}", name=f"pf{h}") for h in range(2)]
            pi = [const.tile([128, 256], bf16, tag=f"pi{h}", name=f"pi{h}") for h in range(2)]
            oa = [const.tile([128, 1], bf16, tag=f"oa{h}", name=f"oa{h}") for h in range(2)]
            ok1 = const.tile([1, 128], bf16, tag="ok1", name="ok1")
            onesb = const.tile([128, 1], bf16, tag="onesb", name="onesb")
            for h in range(2):
                nc.sync.dma_start(out=pf[h][:], in_=pfwd[h])
                nc.sync.dma_start(out=pi[h][:], in_=pinv[h])
                nc.sync.dma_start(out=oa[h][:], in_=onesac[h])
            nc.sync.dma_start(out=ok1[:], in_=onesk)
            nc.sync.dma_start(out=onesb[:], in_=onesk.rearrange("a b -> b a"))
            # zero recon pad regions (head rows + tail rows of each plane)
            for h in range(2):
                base = h * 128 * RSTRIDE
                out_ap = bass.AP(tensor=rh, offset=base,
                                 ap=[[RSTRIDE, 128], [1, 3856]])
                nc.sync.dma_start(out=out_ap, in_=zerosd[:, :3856])
                out_ap = bass.AP(tensor=rh, offset=base + PADL + 137 * 256,
                                 ap=[[RSTRIDE, 128], [1, 4096]])
                nc.sync.dma_start(out=out_ap, in_=zerosd[:, :4096])

            bands = ctx.enter_context(tc.tile_pool(name="bands", bufs=3))
            st = ctx.enter_context(tc.tile_pool(name="st", bufs=2))
            sk = ctx.enter_context(tc.tile_pool(name="sk", bufs=2))
            sw = ctx.enter_context(tc.tile_pool(name="sw", bufs=2))
            sr = ctx.enter_context(tc.tile_pool(name="sr", bufs=2))
            fg = ctx.enter_context(tc.tile_pool(name="fg", bufs=2))
            psc = ctx.enter_context(tc.tile_pool(name="psc", bufs=1, space="PSUM"))
            psn = ctx.enter_context(tc.tile_pool(name="psn", bufs=1, space="PSUM"))
            psw = ctx.enter_context(tc.tile_pool(name="psw", bufs=1, space="PSUM"))
            psr = ctx.enter_context(tc.tile_pool(name="psr", bufs=1, space="PSUM"))
            psf = ctx.enter_context(tc.tile_pool(name="psf", bufs=2, space="PSUM"))

            band_tiles = {}

            def load_group(j):
                nb = min(4, NBAND - 4 * j)
                bt = bands.tile([128, 2048], f16, tag="band", name=f"band{j}")
                in_ap = bass.AP(
                    tensor=xh, offset=4 * j * 512,
                    ap=[[256, 8], [1, 16], [512, nb], [1, 512]])
                nc.sync.dma_start(out=bt[:, :nb * 512], in_=in_ap)
                band_tiles[j] = bt

            load_group(0)
            load_group(1)

            def fold_group(k):
                npair = min(4, FPAIR - 4 * k)
                gt = []
                for h in range(2):
                    g = fg.tile([128, 2048], bf16, tag=f"g{h}", name=f"g{h}_{k}")
                    in_ap = bass.AP(
                        tensor=rh,
                        offset=h * 128 * RSTRIDE + PADL
                        + (8 * k + 15 - 8 * h) * 256,
                        ap=[[16 * RSTRIDE - 256, 8], [RSTRIDE - 1, 16],
                            [512, npair], [1, 512]])
                    nc.sync.dma_start(out=g[:, :npair * 512], in_=in_ap)
                    gt.append(g)
                cvt = fg.tile([1, 2048], f32, tag="cv", name=f"cv{k}")
                for r in range(npair):
                    pF = psf.tile([1, 512], f32, tag="psF", name=f"psF{k}_{r}")
                    for h in range(2):
                        nc.tensor.matmul(pF[:], lhsT=onesb[:, 0:1],
                                         rhs=gt[h][:, r * 512:(r + 1) * 512],
                                         start=(h == 0), stop=(h == 1))
                    nc.scalar.copy(out=cvt[:, r * 512:(r + 1) * 512], in_=pF[:])
                nc.sync.dma_start(
                    out=canvas[None, 4 * k * 512:(4 * k + npair) * 512],
                    in_=cvt[:, :npair * 512])

            wstate = {}
            for t in range(NPAIR):
                if t % 4 == 0 and t // 4 + 2 < NGRP:
                    load_group(t // 4 + 2)
                if t % 8 == 0:
                    smc = st.tile([1, 4096], bf16, tag="smc", name=f"smc{t}")
                    nend = min((t + 8) * 512, NROWS * 256)
                    nc.sync.dma_start(out=smc[:, :nend - t * 512],
                                      in_=seedd[None, t * 512:nend])
                    woc = st.tile([1, 4096], bf16, tag="woc", name=f"woc{t}")
                    wstate["smc"], wstate["woc"], wstate["base"] = smc, woc, t
                off = (t - wstate["base"]) * 512

                # forward DCT: psC[m] = P[:,m-chunk]^T @ patches
                g0, s0 = t // 4, (t % 4) * 512
                g1 = g0 + 1
                pat = [band_tiles[g0][:, s0:s0 + 512],
                       band_tiles[g1][:, s0:s0 + 512]]
                psC = []
                for m in range(2):
                    pc = psc.tile([128, 512], f32, tag=f"psC{m}", name=f"psC{m}_{t}")
                    for h in range(2):
                        nc.tensor.matmul(
                            pc[:],
                            lhsT=pf[h][:, m * 128:(m + 1) * 128],
                            rhs=pat[h],
                            start=(h == 0), stop=(h == 1))
                    psC.append(pc)
                # indicator |c|>thr in one fused op per chunk (Pool)
                ind = []
                for m in range(2):
                    it = sk.tile([128, 512], bf16, tag=f"ind{m}", name=f"ind{m}_{t}")
                    nc.gpsimd.tensor_scalar(
                        out=it[:], in0=psC[m][:], scalar1=0.0, scalar2=thr,
                        op0=Alu.abs_max, op1=Alu.is_gt)
                    ind.append(it)
                # pN = seedrow + sum_AC ind  (seedrow = 1 valid / 1e30 masked)
                pN = psn.tile([1, 512], f32, tag="psN", name=f"psN{t}")
                nc.tensor.matmul(pN[:], lhsT=ok1[:, 0:1],
                                 rhs=wstate["smc"][:, off:off + 512],
                                 start=True, stop=False)
                for m in range(2):
                    nc.tensor.matmul(
                        pN[:], lhsT=oa[m][:, 0:1], rhs=ind[m][:],
                        start=False, stop=(m == 1))
                # w row = 1/pN (mask baked into seed)
                wf = wstate["woc"][:, off:off + 512]
                with nc.allow_low_precision(reason="w weights tolerate bf16"):
                    nc.vector.reciprocal(out=wf, in_=pN[:])
                if t % 8 == 7 or t == NPAIR - 1:
                    nc.sync.dma_start(
                        out=woutd[None, wstate["base"] * 512:(t + 1) * 512],
                        in_=wstate["woc"][:, :off + 512])
                # broadcast w to 128 partitions (K=1 matmul) + bf16 evac
                wbp = psw.tile([128, 512], f32, tag="wbp", name=f"wbp{t}")
                nc.tensor.matmul(wbp[:], lhsT=ok1[:], rhs=wf,
                                 start=True, stop=True)
                wbs = sw.tile([128, 512], bf16, tag="wbs", name=f"wbs{t}")
                nc.scalar.copy(out=wbs[:], in_=wbp[:])
                # shrunk coeffs = psC * ind  (chunk 0 on Pool, chunk 1 on DVE)
                vv = []
                for m in range(2):
                    vt = sk.tile([128, 512], bf16, tag=f"v{m}", name=f"v{m}_{t}")
                    eng = nc.gpsimd if m == 0 else nc.vector
                    eng.scalar_tensor_tensor(
                        out=vt[:], in0=psC[m][:], scalar=0.0, in1=ind[m][:],
                        op0=Alu.add, op1=Alu.mult)
                    vv.append(vt)
                # inverse DCT + w scaling (DVE), both h-chunks into one tile
                rbt = sr.tile([128, 1024], bf16, tag="rb", name=f"rb{t}")
                for h in range(2):
                    pr = psr.tile([128, 512], f32, tag=f"psR{h}", name=f"psR{h}_{t}")
                    for m in range(2):
                        nc.tensor.matmul(
                            pr[:],
                            lhsT=pi[m][:, h * 128:(h + 1) * 128],
                            rhs=vv[m][:],
                            start=(m == 0), stop=(m == 1))
                    nc.vector.tensor_tensor(
                        out=rbt[:, h * 512:(h + 1) * 512], in0=pr[:],
                        in1=wbs[:], op=Alu.mult)
                # single recon writeback for both chunks
                out_ap = bass.AP(
                    tensor=rh, offset=PADL + (2 * t + 15) * 256,
                    ap=[[RSTRIDE, 128], [128 * RSTRIDE, 2], [1, 512]])
                nc.gpsimd.dma_start(out=out_ap, in_=rbt[:])
                if t >= 12 and (t - 12) % 4 == 0:
                    fold_group((t - 12) // 4)
            for k in range(13, (FPAIR + 3) // 4):
                fold_group(k)

    nc.compile()
    return nc


def _prep_inputs(x, Pm):
    """Per-core input maps."""
    Pm = np.ascontiguousarray(Pm, dtype=np.float32)
    pfwd = np.stack([Pm[0:128], Pm[128:256]]).astype(np.float16)
    Pt = np.ascontiguousarray(Pm.T)
    pinv = np.stack([Pt[0:128], Pt[128:256]]).astype(ml_dtypes.bfloat16)
    onesac = np.ones((2, 128, 1), ml_dtypes.bfloat16)
    onesac[0, 0, 0] = 0.0
    in_maps = []
    for core in range(8):
        n, half = core // 2, core % 2
        r0 = 0 if half == 0 else 120
        ximg = np.zeros((NIN, 256), np.float16)
        src = x[n, 0, r0:min(r0 + NIN, 256)]
        ximg[: src.shape[0]] = src.astype(np.float16)
        vrow = 120 if half == 0 else 121
        seed = np.full((NROWS, 256), 1e30, np.float32)
        seed[0:vrow, :Wo] = 1.0
        in_maps.append({
            "ximg": ximg.reshape(-1),
            "pfwd": pfwd, "pinv": pinv, "onesac": onesac,
            "seedd": seed.reshape(-1).astype(ml_dtypes.bfloat16),
            "zeros": np.zeros((128, 4096), ml_dtypes.bfloat16),
            "onesk": np.ones((1, 128), ml_dtypes.bfloat16),
        })
    return in_maps


def _assemble(results, x):
    N = x.shape[0]
    out = np.zeros((N, 256, 256), np.float32)
    wplane = np.zeros((N, 256, 256), np.float32)
    for core in range(8):
        n, half = core // 2, core % 2
        r0 = 0 if half == 0 else 120
        canvas = np.asarray(results[core]["canvas"], np.float32).reshape(-1, 256)
        wout = np.asarray(results[core]["wout"]).astype(np.float32).reshape(NROWS, 256)
        rows = min(canvas.shape[0], 256 - r0)
        out[n, r0:r0 + rows] += canvas[:rows]
        prow = min(NROWS, Ho - r0)
        wplane[n, r0:r0 + prow, :Wo] += wout[:prow, :Wo]
    # divisor: 16x16 box-filter of wplane via 2D cumsum
    cp = np.zeros((N, 257, 257), np.float32)
    cp[:, 1:, 1:] = np.cumsum(np.cumsum(wplane, axis=1), axis=2)
    r1 = np.arange(256) + 1
    r0_ = np.maximum(r1 - PATCH, 0)
    div = (cp[:, r1][:, :, r1] - cp[:, r0_][:, :, r1]
           - cp[:, r1][:, :, r0_] + cp[:, r0_][:, :, r0_])
    return (out / div).reshape(N, 1, 256, 256).astype(np.float32)


def kernel(x, P=None, sigma=None, **_unused):
    from concourse.bass_utils import run_bass_kernel_spmd

    x = np.asarray(x, dtype=np.float32)
    if P is None:
        P = _build_dct_matrix(PATCH)
    P = np.asarray(P, dtype=np.float32)
    sig = float(np.float32(sigma)) if sigma is not None else 0.1
    thr = float(np.float32(3.0) * np.float32(sig))

    key = ("prog", thr)
    if key not in _CACHE:
        _CACHE[key] = _build_program(thr)
    nc = _CACHE[key]

    in_maps = _prep_inputs(x, P)
    trace = os.environ.get("DCT_TRACE") == "1"
    res = run_bass_kernel_spmd(nc, in_maps, list(range(8)), trace=trace)
    global LAST_EXEC_NS
    if res.exec_time_ns is not None:
        LAST_EXEC_NS = res.exec_time_ns
    return _assemble(res.results, x)


if __name__ == "__main__":
    import reference
    inputs = reference.setup_inputs()
    expected = np.asarray(reference.reference(**inputs))
    actual = kernel(**{k: np.asarray(v) for k, v in inputs.items()})
    d = actual - expected
    print("l2 rel:", np.linalg.norm(d) / np.linalg.norm(expected))
    print("max abs:", np.abs(d).max())


# revision 5
# speedup vs baseline: 1.2683x; 1.0960x over previous
"""DCT patch denoiser on 8 Trainium2 NeuronCores.

Sharding: data-parallel over (image, top/bottom half) = 8 shards.
Per core, software-pipelined over 512-patch tiles (stages A/B/C):
  A(t):   fwd DCT (fp16 matmuls from deduped band tiles) -> psC,
          fused indicator |c|>thr (abs_max+is_gt, Pool)
  B(t-1): count = seedrow + ones-matmuls (PE), w = reciprocal (DVE, bf16),
          shrunk coeffs vv = psC*ind (Pool/DVE)
  C(t-2): w broadcast (gpsimd partition_broadcast), inverse DCT (bf16
          matmuls), rb = psR*w (DVE), recon writeback (ACT DMA)
Fold: prefetched batched diagonal-AP gathers (SP DMA), ones-matmul
overlap-add, PSUM->SBUF evac (ACT), canvas writeback.  The divisor
plane (fold of w) and final division happen on host from wout.
"""

import os
import sys
import numpy as np

for _p in ("/opt/trn_rl_repo",):
    if _p not in sys.path:
        sys.path.insert(0, _p)

import ml_dtypes  # noqa: E402

# ---- hardcoded problem geometry ----
PATCH = 16
H = W = 256
Ho = Wo = H - PATCH + 1          # 241
NROWS = 122                       # local patch rows per core (incl masked)
NIN = 138                         # input rows per core
NPAIR = NROWS // 2                # 61 main tiles
FPAIR = 69                        # fold row-pairs -> canvas rows 0..137
PADL = 16                         # head pad elems in recon rows
RSLOT = 153                       # recon row slots (rp+15) in [0,152]
RSTRIDE = PADL + RSLOT * 256      # per-feature stride in recon buffer
NBAND = 65                        # deduped 8-row bands per core
NGRP = 17                         # band groups of <=4
NFG = (FPAIR + 3) // 4            # fold gather groups (18)

_CACHE = {}
LAST_EXEC_NS = None


def _build_dct_matrix(p):
    x = np.arange(p)[:, None]
    i = np.arange(p)[None, :]
    A = np.sqrt(2.0 / p) * np.cos((2 * x + 1) * i * np.pi / (2 * p))
    A[:, 0] /= np.sqrt(2.0)
    return np.kron(A, A).astype(np.float32)


def _fold_schedule():
    """iter -> (pairs list, gather groups list); main iters 0..62 then tail."""
    pair_iter = {}
    for p in range(53):
        pair_iter.setdefault(p + 10, []).append(p)
    ep = [(63, (53, 54)), (64, (55, 56)), (65, (57, 58)), (66, (59, 60)),
          (67, (61, 62)), (68, (63, 64)), (69, (65, 66)), (70, (67, 68))]
    for it, ps in ep:
        pair_iter[it] = list(ps)
    gather_iter = {}
    for k in range(14):
        gather_iter.setdefault(4 * k + 8, []).append(k)
    for k, it in ((14, 63), (15, 64), (16, 66), (17, 68)):
        gather_iter.setdefault(it, []).append(k)
    return pair_iter, gather_iter


def _build_program(thr):
    import concourse.bass as bass
    import concourse.mybir as mybir
    import concourse.tile as tile
    from concourse import bacc
    from contextlib import ExitStack

    dt = mybir.dt
    f32, bf16, f16 = dt.float32, dt.bfloat16, dt.float16
    Alu = mybir.AluOpType

    nc = bacc.Bacc("TRN2", target_bir_lowering=False, debug=False)
    ximg = nc.dram_tensor("ximg", [NIN * 256], f16, kind="ExternalInput").ap()
    pfwd = nc.dram_tensor("pfwd", [2, 128, 256], f16, kind="ExternalInput").ap()
    pinv = nc.dram_tensor("pinv", [2, 128, 256], bf16, kind="ExternalInput").ap()
    onesac = nc.dram_tensor("onesac", [2, 128, 1], bf16, kind="ExternalInput").ap()
    seedd = nc.dram_tensor("seedd", [NROWS * 256], bf16, kind="ExternalInput").ap()
    zerosd = nc.dram_tensor("zeros", [128, 4096], bf16, kind="ExternalInput").ap()
    onesk = nc.dram_tensor("onesk", [1, 128], bf16, kind="ExternalInput").ap()
    canvas = nc.dram_tensor("canvas", [FPAIR * 512], f32, kind="ExternalOutput").ap()
    woutd = nc.dram_tensor("wout", [NROWS * 256], bf16, kind="ExternalOutput").ap()
    recon = nc.dram_tensor("recon", [256 * RSTRIDE], bf16)

    xh = ximg.tensor
    rh = recon[:].tensor

    with tile.TileContext(nc) as tc:
        with ExitStack() as ctx:
            const = ctx.enter_context(tc.tile_pool(name="const", bufs=1))
            pf = [const.tile([128, 256], f16, tag=f"pf{h}", name=f"pf{h}") for h in range(2)]
            pi = [const.tile([128, 256], bf16, tag=f"pi{h}", name=f"pi{h}") for h in range(2)]
            oa = [const.tile([128, 1], bf16, tag=f"oa{h}", name=f"oa{h}") for h in range(2)]
            ok1 = const.tile([1, 128], bf16, tag="ok1", name="ok1")
            onesb = const.tile([128, 1], bf16, tag="onesb", name="onesb")
            for h in range(2):
                nc.sync.dma_start(out=pf[h][:], in_=pfwd[h])
                nc.sync.dma_start(out=pi[h][:], in_=pinv[h])
                nc.sync.dma_start(out=oa[h][:], in_=onesac[h])
            nc.sync.dma_start(out=ok1[:], in_=onesk)
            nc.sync.dma_start(out=onesb[:], in_=onesk.rearrange("a b -> b a"))
            # zero recon pad regions (head rows + tail rows of each plane)
            for h in range(2):
                base = h * 128 * RSTRIDE
                out_ap = bass.AP(tensor=rh, offset=base,
                                 ap=[[RSTRIDE, 128], [1, 3856]])
                nc.sync.dma_start(out=out_ap, in_=zerosd[:, :3856])
                out_ap = bass.AP(tensor=rh, offset=base + PADL + 137 * 256,
                                 ap=[[RSTRIDE, 128], [1, 4096]])
                nc.sync.dma_start(out=out_ap, in_=zerosd[:, :4096])

            bands = ctx.enter_context(tc.tile_pool(name="bands", bufs=3))
            st = ctx.enter_context(tc.tile_pool(name="st", bufs=2))
            sk = ctx.enter_context(tc.tile_pool(name="sk", bufs=2))
            sw = ctx.enter_context(tc.tile_pool(name="sw", bufs=2))
            sr = ctx.enter_context(tc.tile_pool(name="sr", bufs=2))
            fg = ctx.enter_context(tc.tile_pool(name="fg", bufs=2))
            psc = ctx.enter_context(tc.tile_pool(name="psc", bufs=3, space="PSUM"))
            psn = ctx.enter_context(tc.tile_pool(name="psn", bufs=1, space="PSUM"))
            psr = ctx.enter_context(tc.tile_pool(name="psr", bufs=1, space="PSUM"))
            psf = ctx.enter_context(tc.tile_pool(name="psf", bufs=2, space="PSUM"))

            band_tiles = {}

            def load_group(j):
                nb = min(4, NBAND - 4 * j)
                bt = bands.tile([128, 2048], f16, tag="band", name=f"band{j}")
                in_ap = bass.AP(
                    tensor=xh, offset=4 * j * 512,
                    ap=[[256, 8], [1, 16], [512, nb], [1, 512]])
                nc.sync.dma_start(out=bt[:, :nb * 512], in_=in_ap)
                band_tiles[j] = bt

            load_group(0)
            load_group(1)

            # per-tile state carried between pipeline stages
            S = {}
            wblk = {}

            def stage_a(t):
                if t % 4 == 0 and t // 4 + 2 < NGRP:
                    load_group(t // 4 + 2)
                if t % 8 == 0:
                    smc = st.tile([1, 4096], bf16, tag="smc", name=f"smc{t}")
                    nend = min((t + 8) * 512, NROWS * 256)
                    nc.sync.dma_start(out=smc[:, :nend - t * 512],
                                      in_=seedd[None, t * 512:nend])
                    woc = st.tile([1, 4096], bf16, tag="woc", name=f"woc{t}")
                    wblk[t // 8] = (smc, woc)
                g0, s0 = t // 4, (t % 4) * 512
                pat = [band_tiles[g0][:, s0:s0 + 512],
                       band_tiles[g0 + 1][:, s0:s0 + 512]]
                psC = []
                for m in range(2):
                    pc = psc.tile([128, 512], f32, tag="psC", name=f"psC{m}_{t}")
                    for h in range(2):
                        nc.tensor.matmul(
                            pc[:],
                            lhsT=pf[h][:, m * 128:(m + 1) * 128],
                            rhs=pat[h],
                            start=(h == 0), stop=(h == 1))
                    psC.append(pc)
                ind = []
                for m in range(2):
                    it_ = sk.tile([128, 512], bf16, tag=f"ind{m}", name=f"ind{m}_{t}")
                    nc.gpsimd.tensor_scalar(
                        out=it_[:], in0=psC[m][:], scalar1=0.0, scalar2=thr,
                        op0=Alu.abs_max, op1=Alu.is_gt)
                    ind.append(it_)
                S[t] = {"psC": psC, "ind": ind}

            def stage_b(t):
                smc, woc = wblk[t // 8]
                off = (t % 8) * 512
                s = S[t]
                pN = psn.tile([1, 512], f32, tag="psN", name=f"psN{t}")
                nc.tensor.matmul(pN[:], lhsT=ok1[:, 0:1],
                                 rhs=smc[:, off:off + 512],
                                 start=True, stop=False)
                for m in range(2):
                    nc.tensor.matmul(
                        pN[:], lhsT=oa[m][:, 0:1], rhs=s["ind"][m][:],
                        start=False, stop=(m == 1))
                vv = []
                for m in range(2):
                    vt = sk.tile([128, 512], bf16, tag=f"v{m}", name=f"v{m}_{t}")
                    eng = nc.gpsimd if m == 0 else nc.vector
                    eng.scalar_tensor_tensor(
                        out=vt[:], in0=s["psC"][m][:], scalar=0.0,
                        in1=s["ind"][m][:], op0=Alu.add, op1=Alu.mult)
                    vv.append(vt)
                s["vv"] = vv
                wf = woc[:, off:off + 512]
                with nc.allow_low_precision(reason="w weights tolerate bf16"):
                    nc.vector.reciprocal(out=wf, in_=pN[:])
                if t % 8 == 7 or t == NPAIR - 1:
                    base = (t // 8) * 8
                    nc.sync.dma_start(
                        out=woutd[None, base * 512:(t + 1) * 512],
                        in_=woc[:, :(t - base + 1) * 512])
                s["wf"] = wf

            def stage_c(t):
                s = S.pop(t)
                wbs = sw.tile([128, 512], bf16, tag="wbs", name=f"wbs{t}")
                nc.gpsimd.partition_broadcast(wbs[:], s["wf"])
                vv = s["vv"]
                rbt = sr.tile([128, 1024], bf16, tag="rb", name=f"rb{t}")
                for h in range(2):
                    pr = psr.tile([128, 512], f32, tag=f"psR{h}", name=f"psR{h}_{t}")
                    for m in range(2):
                        nc.tensor.matmul(
                            pr[:],
                            lhsT=pi[m][:, h * 128:(h + 1) * 128],
                            rhs=vv[m][:],
                            start=(m == 0), stop=(m == 1))
                    nc.vector.tensor_tensor(
                        out=rbt[:, h * 512:(h + 1) * 512], in0=pr[:],
                        in1=wbs[:], op=Alu.mult)
                out_ap = bass.AP(
                    tensor=rh, offset=PADL + (2 * t + 15) * 256,
                    ap=[[RSTRIDE, 128], [128 * RSTRIDE, 2], [1, 512]])
                nc.scalar.dma_start(out=out_ap, in_=rbt[:])

            gather_tiles = {}

            def fold_gather(k):
                npair = min(4, FPAIR - 4 * k)
                gt = []
                for h in range(2):
                    g = fg.tile([128, 2048], bf16, tag=f"g{h}", name=f"g{h}_{k}")
                    in_ap = bass.AP(
                        tensor=rh,
                        offset=h * 128 * RSTRIDE + PADL
                        + (8 * k + 15 - 8 * h) * 256,
                        ap=[[16 * RSTRIDE - 256, 8], [RSTRIDE - 1, 16],
                            [512, npair], [1, 512]])
                    nc.sync.dma_start(out=g[:, :npair * 512], in_=in_ap)
                    gt.append(g)
                gather_tiles[k] = gt

            cvstate = {"cv": None, "base": 0}

            def fold_pair(p):
                k, r = p // 4, p % 4
                if cvstate["cv"] is None:
                    cvstate["cv"] = fg.tile([1, 2048], f32, tag="cv",
                                            name=f"cv{p}")
                    cvstate["base"] = p
                gt = gather_tiles[k]
                pF = psf.tile([1, 512], f32, tag="psF", name=f"psF{p}")
                for h in range(2):
                    nc.tensor.matmul(pF[:], lhsT=onesb[:, 0:1],
                                     rhs=gt[h][:, r * 512:(r + 1) * 512],
                                     start=(h == 0), stop=(h == 1))
                off = (p - cvstate["base"]) * 512
                nc.scalar.copy(out=cvstate["cv"][:, off:off + 512], in_=pF[:])
                if off == 3 * 512 or p == FPAIR - 1:
                    nc.sync.dma_start(
                        out=canvas[None, cvstate["base"] * 512:(p + 1) * 512],
                        in_=cvstate["cv"][:, :off + 512])
                    cvstate["cv"] = None

            pair_iter, gather_iter = _fold_schedule()
            last_it = max(max(pair_iter), NPAIR + 1)
            for it in range(last_it + 1):
                if it < NPAIR:
                    stage_a(it)
                if 0 <= it - 1 < NPAIR:
                    stage_b(it - 1)
                for k in gather_iter.get(it, ()):
                    fold_gather(k)
                if 0 <= it - 2 < NPAIR:
                    stage_c(it - 2)
                for p in pair_iter.get(it, ()):
                    fold_pair(p)

    nc.compile()
    return nc


def _prep_inputs(x, Pm):
    """Per-core input maps."""
    Pm = np.ascontiguousarray(Pm, dtype=np.float32)
    pfwd = np.stack([Pm[0:128], Pm[128:256]]).astype(np.float16)
    Pt = np.ascontiguousarray(Pm.T)
    pinv = np.stack([Pt[0:128], Pt[128:256]]).astype(ml_dtypes.bfloat16)
    onesac = np.ones((2, 128, 1), ml_dtypes.bfloat16)
    onesac[0, 0, 0] = 0.0
    in_maps = []
    for core in range(8):
        n, half = core // 2, core % 2
        r0 = 0 if half == 0 else 120
        ximg = np.zeros((NIN, 256), np.float16)
        src = x[n, 0, r0:min(r0 + NIN, 256)]
        ximg[: src.shape[0]] = src.astype(np.float16)
        vrow = 120 if half == 0 else 121
        seed = np.full((NROWS, 256), 1e30, np.float32)
        seed[0:vrow, :Wo] = 1.0
        in_maps.append({
            "ximg": ximg.reshape(-1),
            "pfwd": pfwd, "pinv": pinv, "onesac": onesac,
            "seedd": seed.reshape(-1).astype(ml_dtypes.bfloat16),
            "zeros": np.zeros((128, 4096), ml_dtypes.bfloat16),
            "onesk": np.ones((1, 128), ml_dtypes.bfloat16),
        })
    return in_maps


def _assemble(results, x):
    N = x.shape[0]
    out = np.zeros((N, 256, 256), np.float32)
    wplane = np.zeros((N, 256, 256), np.float32)
    for core in range(8):
        n, half = core // 2, core % 2
        r0 = 0 if half == 0 else 120
        canvas = np.asarray(results[core]["canvas"], np.float32).reshape(-1, 256)
        wout = np.asarray(results[core]["wout"]).astype(np.float32).reshape(NROWS, 256)
        rows = min(canvas.shape[0], 256 - r0)
        out[n, r0:r0 + rows] += canvas[:rows]
        prow = min(NROWS, Ho - r0)
        wplane[n, r0:r0 + prow, :Wo] += wout[:prow, :Wo]
    # divisor: 16x16 box-filter of wplane via 2D cumsum
    cp = np.zeros((N, 257, 257), np.float32)
    cp[:, 1:, 1:] = np.cumsum(np.cumsum(wplane, axis=1), axis=2)
    r1 = np.arange(256) + 1
    r0_ = np.maximum(r1 - PATCH, 0)
    div = (cp[:, r1][:, :, r1] - cp[:, r0_][:, :, r1]
           - cp[:, r1][:, :, r0_] + cp[:, r0_][:, :, r0_])
    return (out / div).reshape(N, 1, 256, 256).astype(np.float32)


def kernel(x, P=None, sigma=None, **_unused):
    from concourse.bass_utils import run_bass_kernel_spmd

    x = np.asarray(x, dtype=np.float32)
    if P is None:
        P = _build_dct_matrix(PATCH)
    P = np.asarray(P, dtype=np.float32)
    sig = float(np.float32(sigma)) if sigma is not None else 0.1
    thr = float(np.float32(3.0) * np.float32(sig))

    key = ("prog", thr)
    if key not in _CACHE:
        _CACHE[key] = _build_program(thr)
    nc = _CACHE[key]

    in_maps = _prep_inputs(x, P)
    trace = os.environ.get("DCT_TRACE") == "1"
    res = run_bass_kernel_spmd(nc, in_maps, list(range(8)), trace=trace)
    global LAST_EXEC_NS
    if res.exec_time_ns is not None:
        LAST_EXEC_NS = res.exec_time_ns
    return _assemble(res.results, x)


if __name__ == "__main__":
    import reference
    inputs = reference.setup_inputs()
    expected = np.asarray(reference.reference(**inputs))
    actual = kernel(**{k: np.asarray(v) for k, v in inputs.items()})
    d = actual - expected
    print("l2 rel:", np.linalg.norm(d) / np.linalg.norm(expected))
    print("max abs:", np.abs(d).max())


# revision 8
# speedup vs baseline: 1.4555x; 1.1475x over previous
"""DCT patch denoiser on 8 Trainium2 NeuronCores.

Sharding: data-parallel over (image, top/bottom half) = 8 shards.
Per core, software-pipelined over 512-patch tiles (stages A/B/C):
  A(t):   fwd DCT (fp16 matmuls from deduped band tiles) -> psC,
          fused indicator |c|>thr (abs_max+is_gt, Pool)
  B(t-1): count = seedrow + ones-matmuls (PE), w = reciprocal (DVE, bf16),
          shrunk coeffs vv = psC*ind (Pool/DVE)
  C(t-2): w broadcast (gpsimd partition_broadcast), inverse DCT (bf16
          matmuls), rb = psR*w (DVE), recon writeback (ACT DMA)
Fold: prefetched batched diagonal-AP gathers (SP DMA), ones-matmul
overlap-add, PSUM->SBUF evac (ACT), canvas writeback.  The divisor
plane (fold of w) and final division happen on host from wout.
"""

import os
import sys
import numpy as np

for _p in ("/opt/trn_rl_repo",):
    if _p not in sys.path:
        sys.path.insert(0, _p)

import ml_dtypes  # noqa: E402

# ---- hardcoded problem geometry ----
PATCH = 16
H = W = 256
Ho = Wo = H - PATCH + 1          # 241
NROWS = 122                       # local patch rows per core (incl masked)
NIN = 138                         # input rows per core
NPAIR = NROWS // 2                # 61 main tiles
FPAIR = 69                        # fold row-pairs -> canvas rows 0..137
PADL = 16                         # head pad elems in recon rows
RSLOT = 153                       # recon row slots (rp+15) in [0,152]
RSTRIDE = PADL + RSLOT * 256      # per-feature stride in recon buffer
NBAND = 65                        # deduped 8-row bands per core
NGRP = 9                          # band groups of <=8
NFG = (FPAIR + 7) // 8            # fold gather groups (9)

_CACHE = {}
LAST_EXEC_NS = None


def _build_dct_matrix(p):
    x = np.arange(p)[:, None]
    i = np.arange(p)[None, :]
    A = np.sqrt(2.0 / p) * np.cos((2 * x + 1) * i * np.pi / (2 * p))
    A[:, 0] /= np.sqrt(2.0)
    return np.kron(A, A).astype(np.float32)


def _fold_schedule():
    """iter -> (pairs list, gather groups list); main iters 0..64 then tail."""
    pair_iter = {}
    for p in range(55):
        pair_iter.setdefault(p + 10, []).append(p)
    for i, p in enumerate(range(55, FPAIR, 2)):
        pair_iter.setdefault(65 + i, []).extend(
            q for q in (p, p + 1) if q < FPAIR)
    gather_iter = {}
    for k in range(14):
        gather_iter.setdefault(4 * k + 8, []).append(k)
    for k, it in ((14, 63), (15, 64), (16, 65), (17, 66)):
        gather_iter.setdefault(it, []).append(k)
    return pair_iter, gather_iter


def _build_program(thr):
    import concourse.bass as bass
    import concourse.mybir as mybir
    import concourse.tile as tile
    from concourse import bacc
    from contextlib import ExitStack

    dt = mybir.dt
    f32, bf16, f16, f8 = dt.float32, dt.bfloat16, dt.float16, dt.float8e4
    Alu = mybir.AluOpType

    nc = bacc.Bacc("TRN2", target_bir_lowering=False, debug=False)
    ximg = nc.dram_tensor("ximg", [NIN * 256], f16, kind="ExternalInput").ap()
    pfwd = nc.dram_tensor("pfwd", [2, 128, 256], f16, kind="ExternalInput").ap()
    pinv = nc.dram_tensor("pinv", [2, 128, 256], bf16, kind="ExternalInput").ap()
    onesac = nc.dram_tensor("onesac", [128, 2, 1], f8, kind="ExternalInput").ap()
    seedd = nc.dram_tensor("seedd", [NROWS * 256], bf16, kind="ExternalInput").ap()
    zerosd = nc.dram_tensor("zeros", [128, 4096], bf16, kind="ExternalInput").ap()
    onesk = nc.dram_tensor("onesk", [1, 128], bf16, kind="ExternalInput").ap()
    canvas = nc.dram_tensor("canvas", [FPAIR * 512], f32, kind="ExternalOutput").ap()
    woutd = nc.dram_tensor("wout", [NROWS * 256], bf16, kind="ExternalOutput").ap()
    recon = nc.dram_tensor("recon", [256 * RSTRIDE], bf16)

    xh = ximg.tensor
    rh = recon[:].tensor

    with tile.TileContext(nc) as tc:
        with ExitStack() as ctx:
            const = ctx.enter_context(tc.tile_pool(name="const", bufs=1))
            pf = [const.tile([128, 256], f16, tag=f"pf{h}", name=f"pf{h}") for h in range(2)]
            pi = [const.tile([128, 256], bf16, tag=f"pi{h}", name=f"pi{h}") for h in range(2)]
            oa = const.tile([128, 2, 1], f8, tag="oa", name="oa")
            ok1 = const.tile([1, 128], bf16, tag="ok1", name="ok1")
            onesb = const.tile([128, 1], bf16, tag="onesb", name="onesb")
            for h in range(2):
                nc.sync.dma_start(out=pf[h][:], in_=pfwd[h])
                nc.sync.dma_start(out=pi[h][:], in_=pinv[h])
            nc.sync.dma_start(out=oa[:], in_=onesac)
            nc.sync.dma_start(out=ok1[:], in_=onesk)
            nc.sync.dma_start(out=onesb[:], in_=onesk.rearrange("a b -> b a"))
            # zero recon pad regions (head rows + tail rows of each plane)
            for h in range(2):
                base = h * 128 * RSTRIDE
                out_ap = bass.AP(tensor=rh, offset=base,
                                 ap=[[RSTRIDE, 128], [1, 3856]])
                nc.sync.dma_start(out=out_ap, in_=zerosd[:, :3856])
                out_ap = bass.AP(tensor=rh, offset=base + PADL + 137 * 256,
                                 ap=[[RSTRIDE, 128], [1, 4096]])
                nc.sync.dma_start(out=out_ap, in_=zerosd[:, :4096])

            bands = ctx.enter_context(tc.tile_pool(name="bands", bufs=3))
            st = ctx.enter_context(tc.tile_pool(name="st", bufs=2))
            sk = ctx.enter_context(tc.tile_pool(name="sk", bufs=2))
            sw = ctx.enter_context(tc.tile_pool(name="sw", bufs=2))
            sr = ctx.enter_context(tc.tile_pool(name="sr", bufs=2))
            fg = ctx.enter_context(tc.tile_pool(name="fg", bufs=2))
            psc = ctx.enter_context(tc.tile_pool(name="psc", bufs=3, space="PSUM"))
            psn = ctx.enter_context(tc.tile_pool(name="psn", bufs=1, space="PSUM"))
            psr = ctx.enter_context(tc.tile_pool(name="psr", bufs=1, space="PSUM"))
            psf = ctx.enter_context(tc.tile_pool(name="psf", bufs=2, space="PSUM"))

            band_tiles = {}

            def load_group(j):
                nb = min(8, NBAND - 8 * j)
                bt = bands.tile([128, 4096], f16, tag="band", name=f"band{j}")
                in_ap = bass.AP(
                    tensor=xh, offset=8 * j * 512,
                    ap=[[256, 8], [1, 16], [512, nb], [1, 512]])
                nc.sync.dma_start(out=bt[:, :nb * 512], in_=in_ap)
                band_tiles[j] = bt

            load_group(0)
            load_group(1)

            # per-tile state carried between pipeline stages
            S = {}
            wblk = {}
            rbstate = {}

            def stage_a(t):
                if t % 8 == 0 and t // 8 + 2 < NGRP:
                    load_group(t // 8 + 2)
                if t % 8 == 0:
                    smc = st.tile([1, 4096], bf16, tag="smc", name=f"smc{t}")
                    nend = min((t + 8) * 512, NROWS * 256)
                    nc.sync.dma_start(out=smc[:, :nend - t * 512],
                                      in_=seedd[None, t * 512:nend])
                    woc = st.tile([1, 4096], bf16, tag="woc", name=f"woc{t}")
                    wblk[t // 8] = (smc, woc)
                pat = [band_tiles[t // 8][:, (t % 8) * 512:(t % 8) * 512 + 512],
                       band_tiles[(t + 4) // 8][:, ((t + 4) % 8) * 512:((t + 4) % 8) * 512 + 512]]
                psC = []
                for m in range(2):
                    pc = psc.tile([128, 512], f32, tag="psC", name=f"psC{m}_{t}")
                    for h in range(2):
                        nc.tensor.matmul(
                            pc[:],
                            lhsT=pf[h][:, m * 128:(m + 1) * 128],
                            rhs=pat[h],
                            start=(h == 0), stop=(h == 1))
                    psC.append(pc)
                ind = sk.tile([128, 2, 512], f8, tag="ind", name=f"ind_{t}")
                for m in range(2):
                    nc.gpsimd.tensor_scalar(
                        out=ind[:, m], in0=psC[m][:], scalar1=0.0, scalar2=thr,
                        op0=Alu.abs_max, op1=Alu.is_gt)
                S[t] = {"psC": psC, "ind": ind}

            def stage_b(t):
                smc, woc = wblk[t // 8]
                off = (t % 8) * 512
                s = S[t]
                pN = psn.tile([1, 512], f32, tag="psN", name=f"psN{t}")
                nc.tensor.matmul(pN[:], lhsT=ok1[:, 0:1],
                                 rhs=smc[:, off:off + 512],
                                 start=True, stop=False)
                nc.tensor.matmul(
                    pN[:], lhsT=oa[:], rhs=s["ind"][:],
                    start=False, stop=True,
                    perf_mode=mybir.MatmulPerfMode.DoubleRow)
                vv = []
                for m in range(2):
                    vt = sk.tile([128, 512], bf16, tag=f"v{m}", name=f"v{m}_{t}")
                    eng = nc.gpsimd if m == 0 else nc.vector
                    eng.scalar_tensor_tensor(
                        out=vt[:], in0=s["psC"][m][:], scalar=0.0,
                        in1=s["ind"][:, m], op0=Alu.add, op1=Alu.mult)
                    vv.append(vt)
                s["vv"] = vv
                wf = woc[:, off:off + 512]
                with nc.allow_low_precision(reason="w weights tolerate bf16"):
                    nc.vector.reciprocal(out=wf, in_=pN[:])
                if t % 8 == 7 or t == NPAIR - 1:
                    base = (t // 8) * 8
                    nc.sync.dma_start(
                        out=woutd[None, base * 512:(t + 1) * 512],
                        in_=woc[:, :(t - base + 1) * 512])
                s["wf"] = wf

            def stage_c(t):
                s = S.pop(t)
                wbs = sw.tile([128, 512], bf16, tag="wbs", name=f"wbs{t}")
                nc.gpsimd.partition_broadcast(wbs[:], s["wf"])
                vv = s["vv"]
                if t % 2 == 0:
                    rbt = sr.tile([128, 2048], bf16, tag="rb", name=f"rb{t}")
                    rbstate["rbt"] = rbt
                else:
                    rbt = rbstate["rbt"]
                lo = (t % 2) * 512
                for h in range(2):
                    pr = psr.tile([128, 512], f32, tag=f"psR{h}", name=f"psR{h}_{t}")
                    for m in range(2):
                        nc.tensor.matmul(
                            pr[:],
                            lhsT=pi[m][:, h * 128:(h + 1) * 128],
                            rhs=vv[m][:],
                            start=(m == 0), stop=(m == 1))
                    dst = rbt[:, h * 1024 + lo:h * 1024 + lo + 512]
                    if h == 0 and t % 2 == 1:
                        nc.gpsimd.scalar_tensor_tensor(
                            out=dst, in0=pr[:], scalar=0.0, in1=wbs[:],
                            op0=Alu.add, op1=Alu.mult)
                    else:
                        nc.vector.tensor_tensor(
                            out=dst, in0=pr[:], in1=wbs[:], op=Alu.mult)
                if t % 2 == 1 or t == NPAIR - 1:
                    t0 = t - (t % 2)
                    n = (t % 2) + 1
                    out_ap = bass.AP(
                        tensor=rh, offset=PADL + (2 * t0 + 15) * 256,
                        ap=[[RSTRIDE, 128], [128 * RSTRIDE, 2], [1, 512 * n]])
                    nc.scalar.dma_start(
                        out=out_ap,
                        in_=bass.AP(tensor=rbt.tensor, offset=rbt[:].offset,
                                    ap=[[2048, 128], [1024, 2], [1, 512 * n]])
                        if n == 1 else rbt[:])

            gather_tiles = {}

            def fold_gather(k):
                npair = min(4, FPAIR - 4 * k)
                gt = []
                for h in range(2):
                    g = fg.tile([128, 2048], bf16, tag=f"g{h}", name=f"g{h}_{k}")
                    in_ap = bass.AP(
                        tensor=rh,
                        offset=h * 128 * RSTRIDE + PADL
                        + (8 * k + 15 - 8 * h) * 256,
                        ap=[[16 * RSTRIDE - 256, 8], [RSTRIDE - 1, 16],
                            [512, npair], [1, 512]])
                    nc.scalar.dma_start(out=g[:, :npair * 512], in_=in_ap)
                    gt.append(g)
                gather_tiles[k] = gt

            cvstate = {"cv": None, "base": 0}

            def fold_pair(p):
                k, r = p // 4, p % 4
                if cvstate["cv"] is None:
                    cvstate["cv"] = fg.tile([1, 4096], f32, tag="cv",
                                            name=f"cv{p}")
                    cvstate["base"] = p
                gt = gather_tiles[k]
                pF = psf.tile([1, 512], f32, tag="psF", name=f"psF{p}")
                for h in range(2):
                    nc.tensor.matmul(pF[:], lhsT=onesb[:, 0:1],
                                     rhs=gt[h][:, r * 512:(r + 1) * 512],
                                     start=(h == 0), stop=(h == 1))
                off = (p - cvstate["base"]) * 512
                nc.scalar.copy(out=cvstate["cv"][:, off:off + 512], in_=pF[:])
                if off == 7 * 512 or p == FPAIR - 1:
                    nc.sync.dma_start(
                        out=canvas[None, cvstate["base"] * 512:(p + 1) * 512],
                        in_=cvstate["cv"][:, :off + 512])
                    cvstate["cv"] = None

            pair_iter, gather_iter = _fold_schedule()
            last_it = max(max(pair_iter), NPAIR + 1)
            for it in range(last_it + 1):
                if it < NPAIR:
                    stage_a(it)
                if 0 <= it - 1 < NPAIR:
                    stage_b(it - 1)
                for k in gather_iter.get(it, ()):
                    fold_gather(k)
                if 0 <= it - 2 < NPAIR:
                    stage_c(it - 2)
                for p in pair_iter.get(it, ()):
                    fold_pair(p)

    nc.compile()
    return nc


def _prep_inputs(x, Pm):
    """Per-core input maps."""
    Pm = np.ascontiguousarray(Pm, dtype=np.float32)
    pfwd = np.stack([Pm[0:128], Pm[128:256]]).astype(np.float16)
    Pt = np.ascontiguousarray(Pm.T)
    pinv = np.stack([Pt[0:128], Pt[128:256]]).astype(ml_dtypes.bfloat16)
    onesac = np.ones((128, 2, 1), ml_dtypes.float8_e4m3)
    onesac[0, 0, 0] = 0.0
    in_maps = []
    for core in range(8):
        n, half = core // 2, core % 2
        r0 = 0 if half == 0 else 120
        ximg = np.zeros((NIN, 256), np.float16)
        src = x[n, 0, r0:min(r0 + NIN, 256)]
        ximg[: src.shape[0]] = src.astype(np.float16)
        vrow = 120 if half == 0 else 121
        seed = np.full((NROWS, 256), 1e30, np.float32)
        seed[0:vrow, :Wo] = 1.0
        in_maps.append({
            "ximg": ximg.reshape(-1),
            "pfwd": pfwd, "pinv": pinv, "onesac": onesac,
            "seedd": seed.reshape(-1).astype(ml_dtypes.bfloat16),
            "zeros": np.zeros((128, 4096), ml_dtypes.bfloat16),
            "onesk": np.ones((1, 128), ml_dtypes.bfloat16),
        })
    return in_maps


def _assemble(results, x):
    N = x.shape[0]
    out = np.zeros((N, 256, 256), np.float32)
    wplane = np.zeros((N, 256, 256), np.float32)
    for core in range(8):
        n, half = core // 2, core % 2
        r0 = 0 if half == 0 else 120
        canvas = np.asarray(results[core]["canvas"], np.float32).reshape(-1, 256)
        wout = np.asarray(results[core]["wout"]).astype(np.float32).reshape(NROWS, 256)
        rows = min(canvas.shape[0], 256 - r0)
        out[n, r0:r0 + rows] += canvas[:rows]
        prow = min(NROWS, Ho - r0)
        wplane[n, r0:r0 + prow, :Wo] += wout[:prow, :Wo]
    # divisor: 16x16 box-filter of wplane via 2D cumsum
    cp = np.zeros((N, 257, 257), np.float32)
    cp[:, 1:, 1:] = np.cumsum(np.cumsum(wplane, axis=1), axis=2)
    r1 = np.arange(256) + 1
    r0_ = np.maximum(r1 - PATCH, 0)
    div = (cp[:, r1][:, :, r1] - cp[:, r0_][:, :, r1]
           - cp[:, r1][:, :, r0_] + cp[:, r0_][:, :, r0_])
    return (out / div).reshape(N, 1, 256, 256).astype(np.float32)


def kernel(x, P=None, sigma=None, **_unused):
    from concourse.bass_utils import run_bass_kernel_spmd

    x = np.asarray(x, dtype=np.float32)
    if P is None:
        P = _build_dct_matrix(PATCH)
    P = np.asarray(P, dtype=np.float32)
    sig = float(np.float32(sigma)) if sigma is not None else 0.1
    thr = float(np.float32(3.0) * np.float32(sig))

    key = ("prog", thr)
    if key not in _CACHE:
        _CACHE[key] = _build_program(thr)
    nc = _CACHE[key]

    in_maps = _prep_inputs(x, P)
    trace = os.environ.get("DCT_TRACE") == "1"
    res = run_bass_kernel_spmd(nc, in_maps, list(range(8)), trace=trace)
    global LAST_EXEC_NS
    if res.exec_time_ns is not None:
        LAST_EXEC_NS = res.exec_time_ns
    return _assemble(res.results, x)


if __name__ == "__main__":
    import reference
    inputs = reference.setup_inputs()
    expected = np.asarray(reference.reference(**inputs))
    actual = kernel(**{k: np.asarray(v) for k, v in inputs.items()})
    d = actual - expected
    print("l2 rel:", np.linalg.norm(d) / np.linalg.norm(expected))
    print("max abs:", np.abs(d).max())


# revision 9
# speedup vs baseline: 1.7728x; 1.2180x over previous
"""DCT patch denoiser on 8 Trainium2 NeuronCores.

Sharding: data-parallel over (image, top/bottom half) = 8 shards.
Per core, software-pipelined over 512-patch tiles (stages A/B/C):
  A(t):   fwd DCT (fp16 matmuls from deduped band tiles) -> psC,
          fused indicator |c|>thr (abs_max+is_gt, Pool)
  B(t-1): count = seedrow + ones-matmuls (PE), w = reciprocal (DVE, bf16),
          shrunk coeffs vv = psC*ind (Pool/DVE)
  C(t-2): w broadcast (gpsimd partition_broadcast), inverse DCT (bf16
          matmuls), rb = psR*w (DVE), recon writeback (ACT DMA)
Fold: prefetched batched diagonal-AP gathers (SP DMA), ones-matmul
overlap-add, PSUM->SBUF evac (ACT), canvas writeback.  The divisor
plane (fold of w) and final division happen on host from wout.
"""

import os
import sys
import numpy as np

for _p in ("/opt/trn_rl_repo",):
    if _p not in sys.path:
        sys.path.insert(0, _p)

import ml_dtypes  # noqa: E402

# ---- hardcoded problem geometry ----
PATCH = 16
H = W = 256
Ho = Wo = H - PATCH + 1          # 241
NROWS = 122                       # local patch rows per core (incl masked)
NIN = 138                         # input rows per core
NPAIR = NROWS // 2                # 61 main tiles
FPAIR = 69                        # fold row-pairs -> canvas rows 0..137
PADL = 16                         # head pad elems in recon rows
RSLOT = 153                       # recon row slots (rp+15) in [0,152]
RSTRIDE = PADL + RSLOT * 256      # per-feature stride in recon buffer
NBAND = 65                        # deduped 8-row bands per core
NGRP = 9                          # band groups of <=8
NFG = (FPAIR + 7) // 8            # fold gather groups (9)

_CACHE = {}
LAST_EXEC_NS = None


def _build_dct_matrix(p):
    x = np.arange(p)[:, None]
    i = np.arange(p)[None, :]
    A = np.sqrt(2.0 / p) * np.cos((2 * x + 1) * i * np.pi / (2 * p))
    A[:, 0] /= np.sqrt(2.0)
    return np.kron(A, A).astype(np.float32)


def _fold_schedule():
    """iter -> (pairs list, gather groups list); main iters 0..64 then tail."""
    pair_iter = {}
    for p in range(55):
        pair_iter.setdefault(p + 10, []).append(p)
    for i, p in enumerate(range(55, FPAIR, 2)):
        pair_iter.setdefault(65 + i, []).extend(
            q for q in (p, p + 1) if q < FPAIR)
    gather_iter = {}
    for k in range(14):
        gather_iter.setdefault(4 * k + 8, []).append(k)
    for k, it in ((14, 63), (15, 64), (16, 65), (17, 66)):
        gather_iter.setdefault(it, []).append(k)
    return pair_iter, gather_iter


def _build_program(thr):
    import concourse.bass as bass
    import concourse.mybir as mybir
    import concourse.tile as tile
    from concourse import bacc
    from contextlib import ExitStack

    dt = mybir.dt
    f32, bf16, f16, f8 = dt.float32, dt.bfloat16, dt.float16, dt.float8e4
    Alu = mybir.AluOpType

    nc = bacc.Bacc("TRN2", target_bir_lowering=False, debug=False)
    ximg = nc.dram_tensor("ximg", [NIN * 256], f16, kind="ExternalInput").ap()
    pfwd = nc.dram_tensor("pfwd", [2, 128, 256], f16, kind="ExternalInput").ap()
    pinv = nc.dram_tensor("pinv", [2, 128, 256], bf16, kind="ExternalInput").ap()
    onesac = nc.dram_tensor("onesac", [128, 2, 2], f8, kind="ExternalInput").ap()
    seedd = nc.dram_tensor("seedd", [NROWS * 256], bf16, kind="ExternalInput").ap()
    zerosd = nc.dram_tensor("zeros", [128, 4096], f8, kind="ExternalInput").ap()
    onesk = nc.dram_tensor("onesk", [1, 128], bf16, kind="ExternalInput").ap()
    canvas = nc.dram_tensor("canvas", [FPAIR * 512], f32, kind="ExternalOutput").ap()
    woutd = nc.dram_tensor("wout", [NROWS * 256], bf16, kind="ExternalOutput").ap()
    recon = nc.dram_tensor("recon", [256 * RSTRIDE], f8)

    xh = ximg.tensor
    rh = recon[:].tensor

    with tile.TileContext(nc) as tc:
        with ExitStack() as ctx:
            const = ctx.enter_context(tc.tile_pool(name="const", bufs=1))
            pf = [const.tile([128, 256], f16, tag=f"pf{h}", name=f"pf{h}") for h in range(2)]
            pi = [const.tile([128, 256], bf16, tag=f"pi{h}", name=f"pi{h}") for h in range(2)]
            oa = const.tile([128, 2, 2], f8, tag="oa", name="oa")
            ok1 = const.tile([1, 128], bf16, tag="ok1", name="ok1")
            onesb = const.tile([128, 1], bf16, tag="onesb", name="onesb")
            for h in range(2):
                nc.sync.dma_start(out=pf[h][:], in_=pfwd[h])
                nc.sync.dma_start(out=pi[h][:], in_=pinv[h])
            nc.sync.dma_start(out=oa[:], in_=onesac)
            nc.sync.dma_start(out=ok1[:], in_=onesk)
            nc.sync.dma_start(out=onesb[:], in_=onesk.rearrange("a b -> b a"))
            # zero recon pad regions (head rows + tail rows of each plane)
            for h in range(2):
                base = h * 128 * RSTRIDE
                out_ap = bass.AP(tensor=rh, offset=base,
                                 ap=[[RSTRIDE, 128], [1, 3856]])
                nc.sync.dma_start(out=out_ap, in_=zerosd[:, :3856])
                out_ap = bass.AP(tensor=rh, offset=base + PADL + 137 * 256,
                                 ap=[[RSTRIDE, 128], [1, 4096]])
                nc.sync.dma_start(out=out_ap, in_=zerosd[:, :4096])

            bands = ctx.enter_context(tc.tile_pool(name="bands", bufs=3))
            st = ctx.enter_context(tc.tile_pool(name="st", bufs=2))
            sk = ctx.enter_context(tc.tile_pool(name="sk", bufs=2))
            sw = ctx.enter_context(tc.tile_pool(name="sw", bufs=2))
            sr = ctx.enter_context(tc.tile_pool(name="sr", bufs=2))
            fg = ctx.enter_context(tc.tile_pool(name="fg", bufs=2))
            psc = ctx.enter_context(tc.tile_pool(name="psc", bufs=3, space="PSUM"))
            psn = ctx.enter_context(tc.tile_pool(name="psn", bufs=1, space="PSUM"))
            psr = ctx.enter_context(tc.tile_pool(name="psr", bufs=1, space="PSUM"))
            psf = ctx.enter_context(tc.tile_pool(name="psf", bufs=2, space="PSUM"))

            band_tiles = {}

            def load_group(j):
                nb = min(8, NBAND - 8 * j)
                bt = bands.tile([128, 4096], f16, tag="band", name=f"band{j}")
                in_ap = bass.AP(
                    tensor=xh, offset=8 * j * 512,
                    ap=[[256, 8], [1, 16], [512, nb], [1, 512]])
                nc.sync.dma_start(out=bt[:, :nb * 512], in_=in_ap)
                band_tiles[j] = bt

            load_group(0)
            load_group(1)

            # per-tile state carried between pipeline stages
            S = {}
            wblk = {}
            rbstate = {}

            def stage_a(t):
                if t % 8 == 0 and t // 8 + 2 < NGRP:
                    load_group(t // 8 + 2)
                if t % 8 == 0:
                    smc = st.tile([1, 4096], bf16, tag="smc", name=f"smc{t}")
                    nend = min((t + 8) * 512, NROWS * 256)
                    nc.sync.dma_start(out=smc[:, :nend - t * 512],
                                      in_=seedd[None, t * 512:nend])
                    woc = st.tile([1, 4096], bf16, tag="woc", name=f"woc{t}")
                    wblk[t // 8] = (smc, woc)
                pat = [band_tiles[t // 8][:, (t % 8) * 512:(t % 8) * 512 + 512],
                       band_tiles[(t + 4) // 8][:, ((t + 4) % 8) * 512:((t + 4) % 8) * 512 + 512]]
                psC = []
                for m in range(2):
                    pc = psc.tile([128, 512], f32, tag="psC", name=f"psC{m}_{t}")
                    for h in range(2):
                        nc.tensor.matmul(
                            pc[:],
                            lhsT=pf[h][:, m * 128:(m + 1) * 128],
                            rhs=pat[h],
                            start=(h == 0), stop=(h == 1))
                    psC.append(pc)
                ind = sk.tile([128, 2, 512], f8, tag="ind", name=f"ind_{t}")
                for m in range(2):
                    nc.gpsimd.tensor_scalar(
                        out=ind[:, m], in0=psC[m][:], scalar1=0.0, scalar2=thr,
                        op0=Alu.abs_max, op1=Alu.is_gt)
                S[t] = {"psC": psC, "ind": ind}

            def stage_b(t):
                smc, woc = wblk[t // 8]
                off = (t % 8) * 512
                s = S[t]
                pN = psn.tile([1, 512], f32, tag="psN", name=f"psN{t}")
                nc.tensor.matmul(pN[:], lhsT=ok1[:, 0:1],
                                 rhs=smc[:, off:off + 512],
                                 start=True, stop=False)
                nc.tensor.matmul(
                    pN[:], lhsT=oa[:, :, 0:1], rhs=s["ind"][:],
                    start=False, stop=True,
                    perf_mode=mybir.MatmulPerfMode.DoubleRow)
                vv = []
                for m in range(2):
                    vt = sk.tile([128, 512], bf16, tag=f"v{m}", name=f"v{m}_{t}")
                    eng = nc.gpsimd if m == 0 else nc.vector
                    eng.scalar_tensor_tensor(
                        out=vt[:], in0=s["psC"][m][:], scalar=0.0,
                        in1=s["ind"][:, m], op0=Alu.add, op1=Alu.mult)
                    vv.append(vt)
                s["vv"] = vv
                wf = woc[:, off:off + 512]
                with nc.allow_low_precision(reason="w weights tolerate bf16"):
                    nc.vector.reciprocal(out=wf, in_=pN[:])
                if t % 8 == 7 or t == NPAIR - 1:
                    base = (t // 8) * 8
                    nc.sync.dma_start(
                        out=woutd[None, base * 512:(t + 1) * 512],
                        in_=woc[:, :(t - base + 1) * 512])
                s["wf"] = wf

            def stage_c(t):
                s = S.pop(t)
                wbs = sw.tile([128, 512], bf16, tag="wbs", name=f"wbs{t}")
                nc.gpsimd.partition_broadcast(wbs[:], s["wf"])
                vv = s["vv"]
                if t % 2 == 0:
                    rbt = sr.tile([128, 2048], f8, tag="rb", name=f"rb{t}")
                    rbstate["rbt"] = rbt
                else:
                    rbt = rbstate["rbt"]
                lo = (t % 2) * 512
                for h in range(2):
                    pr = psr.tile([128, 512], f32, tag=f"psR{h}", name=f"psR{h}_{t}")
                    for m in range(2):
                        nc.tensor.matmul(
                            pr[:],
                            lhsT=pi[m][:, h * 128:(h + 1) * 128],
                            rhs=vv[m][:],
                            start=(m == 0), stop=(m == 1))
                    dst = rbt[:, h * 1024 + lo:h * 1024 + lo + 512]
                    if h == 1:
                        nc.gpsimd.scalar_tensor_tensor(
                            out=dst, in0=pr[:], scalar=0.0, in1=wbs[:],
                            op0=Alu.add, op1=Alu.mult)
                    else:
                        nc.vector.tensor_tensor(
                            out=dst, in0=pr[:], in1=wbs[:], op=Alu.mult)
                if t % 2 == 1 or t == NPAIR - 1:
                    t0 = t - (t % 2)
                    n = (t % 2) + 1
                    out_ap = bass.AP(
                        tensor=rh, offset=PADL + (2 * t0 + 15) * 256,
                        ap=[[RSTRIDE, 128], [128 * RSTRIDE, 2], [1, 512 * n]])
                    nc.scalar.dma_start(
                        out=out_ap,
                        in_=bass.AP(tensor=rbt.tensor, offset=rbt[:].offset,
                                    ap=[[2048, 128], [1024, 2], [1, 512 * n]])
                        if n == 1 else rbt[:])

            gather_tiles = {}

            def fold_gather(k):
                npair = min(4, FPAIR - 4 * k)
                g = fg.tile([128, 2, 2048], f8, tag="g", name=f"g_{k}")
                for h in range(2):
                    in_ap = bass.AP(
                        tensor=rh,
                        offset=h * 128 * RSTRIDE + PADL
                        + (8 * k + 15 - 8 * h) * 256,
                        ap=[[16 * RSTRIDE - 256, 8], [RSTRIDE - 1, 16],
                            [512, npair], [1, 512]])
                    eng = nc.sync if h == 0 else nc.scalar
                    eng.dma_start(out=g[:, h, :npair * 512], in_=in_ap)
                gather_tiles[k] = g

            cvstate = {"cv": None, "base": 0}

            def fold_pair(p):
                k, r = p // 4, p % 4
                if cvstate["cv"] is None:
                    cvstate["cv"] = fg.tile([1, 4096], f32, tag="cv",
                                            name=f"cv{p}")
                    cvstate["base"] = p
                gt = gather_tiles[k]
                pF = psf.tile([1, 512], f32, tag="psF", name=f"psF{p}")
                nc.tensor.matmul(pF[:], lhsT=oa[:, :, 1:2],
                                 rhs=gt[:, :, r * 512:(r + 1) * 512],
                                 start=True, stop=True,
                                 perf_mode=mybir.MatmulPerfMode.DoubleRow)
                off = (p - cvstate["base"]) * 512
                nc.scalar.copy(out=cvstate["cv"][:, off:off + 512], in_=pF[:])
                if off == 7 * 512 or p == FPAIR - 1:
                    nc.sync.dma_start(
                        out=canvas[None, cvstate["base"] * 512:(p + 1) * 512],
                        in_=cvstate["cv"][:, :off + 512])
                    cvstate["cv"] = None

            pair_iter, gather_iter = _fold_schedule()
            last_it = max(max(pair_iter), NPAIR + 1)
            for it in range(last_it + 1):
                if it < NPAIR:
                    stage_a(it)
                if 0 <= it - 1 < NPAIR:
                    stage_b(it - 1)
                for k in gather_iter.get(it, ()):
                    fold_gather(k)
                if 0 <= it - 2 < NPAIR:
                    stage_c(it - 2)
                for p in pair_iter.get(it, ()):
                    fold_pair(p)

    nc.compile()
    return nc


def _prep_inputs(x, Pm):
    """Per-core input maps."""
    Pm = np.ascontiguousarray(Pm, dtype=np.float32)
    pfwd = np.stack([Pm[0:128], Pm[128:256]]).astype(np.float16)
    Pt = np.ascontiguousarray(Pm.T)
    pinv = np.stack([Pt[0:128], Pt[128:256]]).astype(ml_dtypes.bfloat16)
    onesac = np.ones((128, 2, 2), ml_dtypes.float8_e4m3)
    onesac[0, 0, 0] = 0.0
    in_maps = []
    for core in range(8):
        n, half = core // 2, core % 2
        r0 = 0 if half == 0 else 120
        ximg = np.zeros((NIN, 256), np.float16)
        src = x[n, 0, r0:min(r0 + NIN, 256)]
        ximg[: src.shape[0]] = src.astype(np.float16)
        vrow = 120 if half == 0 else 121
        seed = np.full((NROWS, 256), 1e30, np.float32)
        seed[0:vrow, :Wo] = 1.0
        in_maps.append({
            "ximg": ximg.reshape(-1),
            "pfwd": pfwd, "pinv": pinv, "onesac": onesac,
            "seedd": seed.reshape(-1).astype(ml_dtypes.bfloat16),
            "zeros": np.zeros((128, 4096), ml_dtypes.float8_e4m3),
            "onesk": np.ones((1, 128), ml_dtypes.bfloat16),
        })
    return in_maps


def _assemble(results, x):
    N = x.shape[0]
    out = np.zeros((N, 256, 256), np.float32)
    wplane = np.zeros((N, 256, 256), np.float32)
    for core in range(8):
        n, half = core // 2, core % 2
        r0 = 0 if half == 0 else 120
        canvas = np.asarray(results[core]["canvas"], np.float32).reshape(-1, 256)
        wout = np.asarray(results[core]["wout"]).astype(np.float32).reshape(NROWS, 256)
        rows = min(canvas.shape[0], 256 - r0)
        out[n, r0:r0 + rows] += canvas[:rows]
        prow = min(NROWS, Ho - r0)
        wplane[n, r0:r0 + prow, :Wo] += wout[:prow, :Wo]
    # divisor: 16x16 box-filter of wplane via 2D cumsum
    cp = np.zeros((N, 257, 257), np.float32)
    cp[:, 1:, 1:] = np.cumsum(np.cumsum(wplane, axis=1), axis=2)
    r1 = np.arange(256) + 1
    r0_ = np.maximum(r1 - PATCH, 0)
    div = (cp[:, r1][:, :, r1] - cp[:, r0_][:, :, r1]
           - cp[:, r1][:, :, r0_] + cp[:, r0_][:, :, r0_])
    return (out / div).reshape(N, 1, 256, 256).astype(np.float32)


def kernel(x, P=None, sigma=None, **_unused):
    from concourse.bass_utils import run_bass_kernel_spmd

    x = np.asarray(x, dtype=np.float32)
    if P is None:
        P = _build_dct_matrix(PATCH)
    P = np.asarray(P, dtype=np.float32)
    sig = float(np.float32(sigma)) if sigma is not None else 0.1
    thr = float(np.float32(3.0) * np.float32(sig))

    key = ("prog", thr)
    if key not in _CACHE:
        _CACHE[key] = _build_program(thr)
    nc = _CACHE[key]

    in_maps = _prep_inputs(x, P)
    trace = os.environ.get("DCT_TRACE") == "1"
    res = run_bass_kernel_spmd(nc, in_maps, list(range(8)), trace=trace)
    global LAST_EXEC_NS
    if res.exec_time_ns is not None:
        LAST_EXEC_NS = res.exec_time_ns
    return _assemble(res.results, x)


if __name__ == "__main__":
    import reference
    inputs = reference.setup_inputs()
    expected = np.asarray(reference.reference(**inputs))
    actual = kernel(**{k: np.asarray(v) for k, v in inputs.items()})
    d = actual - expected
    print("l2 rel:", np.linalg.norm(d) / np.linalg.norm(expected))
    print("max abs:", np.abs(d).max())


# revision 10
# speedup vs baseline: 1.8763x; 1.0584x over previous
"""DCT patch denoiser on 8 Trainium2 NeuronCores.

Sharding: data-parallel over (image, top/bottom half) = 8 shards.
Per core, software-pipelined over 512-patch tiles (stages A/B/C):
  A(t):   fwd DCT (fp16 matmuls from deduped band tiles) -> psC,
          fused indicator |c|>thr (abs_max+is_gt, Pool)
  B(t-1): count = seedrow + ones-matmuls (PE), w = reciprocal (DVE, bf16),
          shrunk coeffs vv = psC*ind (Pool/DVE)
  C(t-2): w broadcast (gpsimd partition_broadcast), inverse DCT (bf16
          matmuls), rb = psR*w (DVE), recon writeback (ACT DMA)
Fold: prefetched batched diagonal-AP gathers (SP DMA), ones-matmul
overlap-add, PSUM->SBUF evac (ACT), canvas writeback.  The divisor
plane (fold of w) and final division happen on host from wout.
"""

import os
import sys
import numpy as np

for _p in ("/opt/trn_rl_repo",):
    if _p not in sys.path:
        sys.path.insert(0, _p)

import ml_dtypes  # noqa: E402

# ---- hardcoded problem geometry ----
PATCH = 16
H = W = 256
Ho = Wo = H - PATCH + 1          # 241
NROWS = 122                       # local patch rows per core (incl masked)
NIN = 138                         # input rows per core
NPAIR = NROWS // 2                # 61 main tiles
FPAIR = 69                        # fold row-pairs -> canvas rows 0..137
PADL = 16                         # head pad elems in recon rows
RSLOT = 153                       # recon row slots (rp+15) in [0,152]
RSTRIDE = PADL + RSLOT * 256      # per-feature stride in recon buffer
NBAND = 65                        # deduped 8-row bands per core
NGRP = 9                          # band groups of <=8
NFG = (FPAIR + 7) // 8            # fold gather groups (9)

_CACHE = {}
LAST_EXEC_NS = None


def _build_dct_matrix(p):
    x = np.arange(p)[:, None]
    i = np.arange(p)[None, :]
    A = np.sqrt(2.0 / p) * np.cos((2 * x + 1) * i * np.pi / (2 * p))
    A[:, 0] /= np.sqrt(2.0)
    return np.kron(A, A).astype(np.float32)


def _fold_schedule():
    """Greedy fold packing: pair p needs recon of tile min(p,60) (issued at
    iter min(p,60)+2) plus margin; its gather group must be issued >=1 iter
    earlier; gathers can run at most 2 groups ahead (fg ring)."""
    pair_iter, gather_iter = {}, {}
    gather_at = {}
    nextp, nextk = 0, 0
    for it in range(8, 200):
        if nextp >= FPAIR and nextk >= 18:
            break
        # issue gathers: safety margin 4 after recon issue; <=2 groups ahead
        while (nextk < 18
               and it >= min(4 * nextk + 3, NPAIR - 1) + 2 + 4
               and 4 * nextk <= nextp + 7):
            gather_iter.setdefault(it, []).append(nextk)
            gather_at[nextk] = it
            nextk += 1
        cap = 1 if it <= 55 else 2
        for _ in range(cap):
            if nextp >= FPAIR:
                break
            p = nextp
            if (it >= min(p, NPAIR - 1) + 2 + 4
                    and p // 4 in gather_at
                    and it >= gather_at[p // 4] + 2):
                pair_iter.setdefault(it, []).append(p)
                nextp += 1
            else:
                break
    return pair_iter, gather_iter


def _build_program(thr):
    import concourse.bass as bass
    import concourse.mybir as mybir
    import concourse.tile as tile
    from concourse import bacc
    from contextlib import ExitStack

    dt = mybir.dt
    f32, bf16, f16, f8 = dt.float32, dt.bfloat16, dt.float16, dt.float8e4
    Alu = mybir.AluOpType

    nc = bacc.Bacc("TRN2", target_bir_lowering=False, debug=False)
    ximg = nc.dram_tensor("ximg", [NIN * 256], f16, kind="ExternalInput").ap()
    pfwd = nc.dram_tensor("pfwd", [2, 128, 256], f16, kind="ExternalInput").ap()
    pinv = nc.dram_tensor("pinv", [2, 128, 256], bf16, kind="ExternalInput").ap()
    onesac = nc.dram_tensor("onesac", [128, 2, 2], f8, kind="ExternalInput").ap()
    seedd = nc.dram_tensor("seedd", [NROWS * 256], bf16, kind="ExternalInput").ap()
    zerosd = nc.dram_tensor("zeros", [128, 4096], f8, kind="ExternalInput").ap()
    onesk = nc.dram_tensor("onesk", [1, 128], bf16, kind="ExternalInput").ap()
    canvas = nc.dram_tensor("canvas", [FPAIR * 512], f32, kind="ExternalOutput").ap()
    woutd = nc.dram_tensor("wout", [NROWS * 256], bf16, kind="ExternalOutput").ap()
    recon = nc.dram_tensor("recon", [256 * RSTRIDE], f8)

    xh = ximg.tensor
    rh = recon[:].tensor

    with tile.TileContext(nc) as tc:
        with ExitStack() as ctx:
            const = ctx.enter_context(tc.tile_pool(name="const", bufs=1))
            pf = [const.tile([128, 256], f16, tag=f"pf{h}", name=f"pf{h}") for h in range(2)]
            pi = [const.tile([128, 256], bf16, tag=f"pi{h}", name=f"pi{h}") for h in range(2)]
            oa = const.tile([128, 2, 2], f8, tag="oa", name="oa")
            ok1 = const.tile([1, 128], bf16, tag="ok1", name="ok1")
            onesb = const.tile([128, 1], bf16, tag="onesb", name="onesb")
            for h in range(2):
                nc.scalar.dma_start(out=pf[h][:], in_=pfwd[h])
                nc.scalar.dma_start(out=pi[h][:], in_=pinv[h])
            nc.scalar.dma_start(out=oa[:], in_=onesac)
            nc.scalar.dma_start(out=ok1[:], in_=onesk)
            nc.scalar.dma_start(out=onesb[:], in_=onesk.rearrange("a b -> b a"))
            # zero recon pad regions (head rows + tail rows of each plane)
            for h in range(2):
                base = h * 128 * RSTRIDE
                out_ap = bass.AP(tensor=rh, offset=base,
                                 ap=[[RSTRIDE, 128], [1, 3856]])
                nc.gpsimd.dma_start(out=out_ap, in_=zerosd[:, :3856])
                out_ap = bass.AP(tensor=rh, offset=base + PADL + 137 * 256,
                                 ap=[[RSTRIDE, 128], [1, 4096]])
                nc.gpsimd.dma_start(out=out_ap, in_=zerosd[:, :4096])

            bands = ctx.enter_context(tc.tile_pool(name="bands", bufs=3))
            st = ctx.enter_context(tc.tile_pool(name="st", bufs=2))
            sk = ctx.enter_context(tc.tile_pool(name="sk", bufs=2))
            sw = ctx.enter_context(tc.tile_pool(name="sw", bufs=2))
            sr = ctx.enter_context(tc.tile_pool(name="sr", bufs=2))
            fg = ctx.enter_context(tc.tile_pool(name="fg", bufs=2))
            psc = ctx.enter_context(tc.tile_pool(name="psc", bufs=3, space="PSUM"))
            psn = ctx.enter_context(tc.tile_pool(name="psn", bufs=1, space="PSUM"))
            psr = ctx.enter_context(tc.tile_pool(name="psr", bufs=1, space="PSUM"))
            psf = ctx.enter_context(tc.tile_pool(name="psf", bufs=2, space="PSUM"))

            band_tiles = {}

            def load_group(j):
                nb = min(8, NBAND - 8 * j)
                bt = bands.tile([128, 4096], f16, tag="band", name=f"band{j}")
                in_ap = bass.AP(
                    tensor=xh, offset=8 * j * 512,
                    ap=[[256, 8], [1, 16], [512, nb], [1, 512]])
                nc.sync.dma_start(out=bt[:, :nb * 512], in_=in_ap)
                band_tiles[j] = bt

            load_group(0)
            load_group(1)

            # per-tile state carried between pipeline stages
            S = {}
            wblk = {}
            rbstate = {}

            def stage_a(t):
                if t % 8 == 0 and t // 8 + 2 < NGRP:
                    load_group(t // 8 + 2)
                if t % 8 == 0:
                    smc = st.tile([1, 4096], bf16, tag="smc", name=f"smc{t}")
                    nend = min((t + 8) * 512, NROWS * 256)
                    nc.sync.dma_start(out=smc[:, :nend - t * 512],
                                      in_=seedd[None, t * 512:nend])
                    woc = st.tile([1, 4096], bf16, tag="woc", name=f"woc{t}")
                    wblk[t // 8] = (smc, woc)
                pat = [band_tiles[t // 8][:, (t % 8) * 512:(t % 8) * 512 + 512],
                       band_tiles[(t + 4) // 8][:, ((t + 4) % 8) * 512:((t + 4) % 8) * 512 + 512]]
                psC = []
                for m in range(2):
                    pc = psc.tile([128, 512], f32, tag="psC", name=f"psC{m}_{t}")
                    for h in range(2):
                        nc.tensor.matmul(
                            pc[:],
                            lhsT=pf[h][:, m * 128:(m + 1) * 128],
                            rhs=pat[h],
                            start=(h == 0), stop=(h == 1))
                    psC.append(pc)
                ind = sk.tile([128, 2, 512], f8, tag="ind", name=f"ind_{t}")
                for m in range(2):
                    nc.gpsimd.tensor_scalar(
                        out=ind[:, m], in0=psC[m][:], scalar1=0.0, scalar2=thr,
                        op0=Alu.abs_max, op1=Alu.is_gt)
                S[t] = {"psC": psC, "ind": ind}

            def stage_b(t):
                smc, woc = wblk[t // 8]
                off = (t % 8) * 512
                s = S[t]
                pN = psn.tile([1, 512], f32, tag="psN", name=f"psN{t}")
                nc.tensor.matmul(pN[:], lhsT=ok1[:, 0:1],
                                 rhs=smc[:, off:off + 512],
                                 start=True, stop=False)
                nc.tensor.matmul(
                    pN[:], lhsT=oa[:, :, 0:1], rhs=s["ind"][:],
                    start=False, stop=True,
                    perf_mode=mybir.MatmulPerfMode.DoubleRow)
                vv = []
                for m in range(2):
                    vt = sk.tile([128, 512], bf16, tag=f"v{m}", name=f"v{m}_{t}")
                    eng = nc.gpsimd if m == 0 else nc.vector
                    eng.scalar_tensor_tensor(
                        out=vt[:], in0=s["psC"][m][:], scalar=0.0,
                        in1=s["ind"][:, m], op0=Alu.add, op1=Alu.mult)
                    vv.append(vt)
                s["vv"] = vv
                wf = woc[:, off:off + 512]
                with nc.allow_low_precision(reason="w weights tolerate bf16"):
                    nc.vector.reciprocal(out=wf, in_=pN[:])
                if t % 8 == 7 or t == NPAIR - 1:
                    base = (t // 8) * 8
                    nc.sync.dma_start(
                        out=woutd[None, base * 512:(t + 1) * 512],
                        in_=woc[:, :(t - base + 1) * 512])
                s["wf"] = wf

            def stage_c(t):
                s = S.pop(t)
                wbs = sw.tile([128, 512], bf16, tag="wbs", name=f"wbs{t}")
                nc.gpsimd.partition_broadcast(wbs[:], s["wf"])
                vv = s["vv"]
                if t % 2 == 0:
                    rbt = sr.tile([128, 2048], f8, tag="rb", name=f"rb{t}")
                    rbstate["rbt"] = rbt
                else:
                    rbt = rbstate["rbt"]
                lo = (t % 2) * 512
                for h in range(2):
                    pr = psr.tile([128, 512], f32, tag=f"psR{h}", name=f"psR{h}_{t}")
                    for m in range(2):
                        nc.tensor.matmul(
                            pr[:],
                            lhsT=pi[m][:, h * 128:(h + 1) * 128],
                            rhs=vv[m][:],
                            start=(m == 0), stop=(m == 1))
                    dst = rbt[:, h * 1024 + lo:h * 1024 + lo + 512]
                    if h == 1:
                        nc.gpsimd.scalar_tensor_tensor(
                            out=dst, in0=pr[:], scalar=0.0, in1=wbs[:],
                            op0=Alu.add, op1=Alu.mult)
                    else:
                        nc.vector.tensor_tensor(
                            out=dst, in0=pr[:], in1=wbs[:], op=Alu.mult)
                if t % 2 == 1 or t == NPAIR - 1:
                    t0 = t - (t % 2)
                    n = (t % 2) + 1
                    out_ap = bass.AP(
                        tensor=rh, offset=PADL + (2 * t0 + 15) * 256,
                        ap=[[RSTRIDE, 128], [128 * RSTRIDE, 2], [1, 512 * n]])
                    nc.scalar.dma_start(
                        out=out_ap,
                        in_=bass.AP(tensor=rbt.tensor, offset=rbt[:].offset,
                                    ap=[[2048, 128], [1024, 2], [1, 512 * n]])
                        if n == 1 else rbt[:])

            gather_tiles = {}

            def fold_gather(k):
                npair = min(4, FPAIR - 4 * k)
                g = fg.tile([128, 2, 2048], f8, tag="g", name=f"g_{k}")
                for h in range(2):
                    in_ap = bass.AP(
                        tensor=rh,
                        offset=h * 128 * RSTRIDE + PADL
                        + (8 * k + 15 - 8 * h) * 256,
                        ap=[[16 * RSTRIDE - 256, 8], [RSTRIDE - 1, 16],
                            [512, npair], [1, 512]])
                    eng = nc.sync if h == 0 else nc.scalar
                    eng.dma_start(out=g[:, h, :npair * 512], in_=in_ap)
                gather_tiles[k] = g

            cvstate = {"cv": None, "base": 0}

            def fold_pair(p, tail=False):
                k, r = p // 4, p % 4
                if cvstate["cv"] is None:
                    cvstate["cv"] = fg.tile([1, 4096], f32, tag="cv",
                                            name=f"cv{p}")
                    cvstate["base"] = p
                gt = gather_tiles[k]
                pF = psf.tile([1, 512], f32, tag="psF", name=f"psF{p}")
                nc.tensor.matmul(pF[:], lhsT=oa[:, :, 1:2],
                                 rhs=gt[:, :, r * 512:(r + 1) * 512],
                                 start=True, stop=True,
                                 perf_mode=mybir.MatmulPerfMode.DoubleRow)
                off = (p - cvstate["base"]) * 512
                dst = cvstate["cv"][:, off:off + 512]
                if tail and p % 2 == 1:
                    nc.vector.tensor_scalar(out=dst, in0=pF[:], scalar1=0.0,
                                            scalar2=None, op0=Alu.add)
                else:
                    nc.scalar.copy(out=dst, in_=pF[:])
                if off == 7 * 512 or p == FPAIR - 1:
                    nc.sync.dma_start(
                        out=canvas[None, cvstate["base"] * 512:(p + 1) * 512],
                        in_=cvstate["cv"][:, :off + 512])
                    cvstate["cv"] = None

            pair_iter, gather_iter = _fold_schedule()
            last_it = max(max(pair_iter), NPAIR + 1)
            for it in range(last_it + 1):
                if it < NPAIR:
                    stage_a(it)
                if 0 <= it - 1 < NPAIR:
                    stage_b(it - 1)
                for k in gather_iter.get(it, ()):
                    fold_gather(k)
                if 0 <= it - 2 < NPAIR:
                    stage_c(it - 2)
                for p in pair_iter.get(it, ()):
                    fold_pair(p, tail=it > NPAIR + 1)

    nc.compile()
    return nc


def _prep_inputs(x, Pm):
    """Per-core input maps."""
    Pm = np.ascontiguousarray(Pm, dtype=np.float32)
    pfwd = np.stack([Pm[0:128], Pm[128:256]]).astype(np.float16)
    Pt = np.ascontiguousarray(Pm.T)
    pinv = np.stack([Pt[0:128], Pt[128:256]]).astype(ml_dtypes.bfloat16)
    onesac = np.ones((128, 2, 2), ml_dtypes.float8_e4m3)
    onesac[0, 0, 0] = 0.0
    in_maps = []
    for core in range(8):
        n, half = core // 2, core % 2
        r0 = 0 if half == 0 else 120
        ximg = np.zeros((NIN, 256), np.float16)
        src = x[n, 0, r0:min(r0 + NIN, 256)]
        ximg[: src.shape[0]] = src.astype(np.float16)
        vrow = 120 if half == 0 else 121
        seed = np.full((NROWS, 256), 1e30, np.float32)
        seed[0:vrow, :Wo] = 1.0
        in_maps.append({
            "ximg": ximg.reshape(-1),
            "pfwd": pfwd, "pinv": pinv, "onesac": onesac,
            "seedd": seed.reshape(-1).astype(ml_dtypes.bfloat16),
            "zeros": np.zeros((128, 4096), ml_dtypes.float8_e4m3),
            "onesk": np.ones((1, 128), ml_dtypes.bfloat16),
        })
    return in_maps


def _assemble(results, x):
    N = x.shape[0]
    out = np.zeros((N, 256, 256), np.float32)
    wplane = np.zeros((N, 256, 256), np.float32)
    for core in range(8):
        n, half = core // 2, core % 2
        r0 = 0 if half == 0 else 120
        canvas = np.asarray(results[core]["canvas"], np.float32).reshape(-1, 256)
        wout = np.asarray(results[core]["wout"]).astype(np.float32).reshape(NROWS, 256)
        rows = min(canvas.shape[0], 256 - r0)
        out[n, r0:r0 + rows] += canvas[:rows]
        prow = min(NROWS, Ho - r0)
        wplane[n, r0:r0 + prow, :Wo] += wout[:prow, :Wo]
    # divisor: 16x16 box-filter of wplane via 2D cumsum
    cp = np.zeros((N, 257, 257), np.float32)
    cp[:, 1:, 1:] = np.cumsum(np.cumsum(wplane, axis=1), axis=2)
    r1 = np.arange(256) + 1
    r0_ = np.maximum(r1 - PATCH, 0)
    div = (cp[:, r1][:, :, r1] - cp[:, r0_][:, :, r1]
           - cp[:, r1][:, :, r0_] + cp[:, r0_][:, :, r0_])
    return (out / div).reshape(N, 1, 256, 256).astype(np.float32)


def kernel(x, P=None, sigma=None, **_unused):
    from concourse.bass_utils import run_bass_kernel_spmd

    x = np.asarray(x, dtype=np.float32)
    if P is None:
        P = _build_dct_matrix(PATCH)
    P = np.asarray(P, dtype=np.float32)
    sig = float(np.float32(sigma)) if sigma is not None else 0.1
    thr = float(np.float32(3.0) * np.float32(sig))

    key = ("prog", thr)
    if key not in _CACHE:
        _CACHE[key] = _build_program(thr)
    nc = _CACHE[key]

    in_maps = _prep_inputs(x, P)
    trace = os.environ.get("DCT_TRACE") == "1"
    res = run_bass_kernel_spmd(nc, in_maps, list(range(8)), trace=trace)
    global LAST_EXEC_NS
    if res.exec_time_ns is not None:
        LAST_EXEC_NS = res.exec_time_ns
    return _assemble(res.results, x)


if __name__ == "__main__":
    import reference
    inputs = reference.setup_inputs()
    expected = np.asarray(reference.reference(**inputs))
    actual = kernel(**{k: np.asarray(v) for k, v in inputs.items()})
    d = actual - expected
    print("l2 rel:", np.linalg.norm(d) / np.linalg.norm(expected))
    print("max abs:", np.abs(d).max())
